# revision 1
# baseline (speedup 1.0000x reference)
"""Batched CRF Viterbi decode (N=64, C=8, L=32768) on 8 TRN2 NeuronCores.

Self-contained kernel: takes FULL unsharded inputs, shards the batch dim
across 8 cores (data-parallel), runs a Bass/Tile kernel per core, and
gathers the full [64, 32768] int32 path.

Algorithm (bit-exact with the fp32 jax reference, including argmax
first-index tie-breaking):
  Phase 1 (serial): the viterbi forward recurrence on the vector engine,
    3 ops/timestep; stores the vit[t] series.
  Phase 2 (parallel): backpointers+1 via first-index-of-max recovered from
    the vit series; end-nodes; the length-1 reset folded into the maps.
  Phase 3 (parallel): chunked backward traversal (integer-exact):
    per-chunk candidate trajectories for all 8 entry states, hierarchical
    map composition for chunk entries, final select + length mask.
"""
import sys
import numpy as np

if '/opt/trn_rl_repo' not in sys.path:
    sys.path.insert(0, '/opt/trn_rl_repo')

N_FULL, C, L = 64, 8, 32768
SEQ = 8          # sequences per core
NSTRIP = 16      # time strips per core (partition dim = NSTRIP*SEQ = 128)
S = 16           # phase-3 chunk length
NCORES = 8

_CACHE = {}


def _shapes(L):
    STRIP = L // NSTRIP
    TB = min(128, STRIP)
    return dict(STRIP=STRIP, TB=TB, ROUNDS=STRIP // TB, KL=STRIP // S,
                BLK=min(2048, L))


def _host_prep(observes_core, transitions, lengths_core, L):
    sh = _shapes(L)
    STRIP, KL = sh["STRIP"], sh["KL"]
    obs_t = np.ascontiguousarray(
        np.transpose(np.asarray(observes_core, np.float32), (0, 2, 1)))
    obs_pad = np.concatenate([np.zeros((SEQ, 1, C), np.float32), obs_t], 1)
    T = np.asarray(transitions, np.float32)
    lens = np.asarray(lengths_core).astype(np.float32)
    p = np.arange(128)
    return {
        "obs": obs_pad.reshape(SEQ, (L + 1) * C),
        "trep": np.tile(T.reshape(1, C * C), (128, 1)).astype(np.float32),
        "wdesc": np.tile((C - np.arange(C, dtype=np.float32)).reshape(1, C), (128, 1)),
        "tplane": ((p[:, None] // SEQ) * STRIP
                   + np.arange(STRIP)[None, :]).astype(np.float32),
        "len_col": lens[p % SEQ][:, None].astype(np.float32),
        "lenm1": (lens[p % SEQ][:, None] - 1.0).astype(np.float32),
        "einit1": np.tile((np.arange(C, dtype=np.float32)[:, None] + 1.0),
                          (1, KL)).reshape(1, C * KL).repeat(128, 0).astype(np.float32),
    }


def _host_post(path_dev, L):
    STRIP = L // NSTRIP
    return path_dev.reshape(NSTRIP, SEQ, STRIP).transpose(1, 0, 2).reshape(SEQ, L)


def _emit(tc, ins, outs, L):
    import concourse.bass as bass
    import concourse.mybir as mybir
    import bass_rust

    F32 = mybir.dt.float32
    I32 = mybir.dt.int32
    ALU = mybir.AluOpType
    AX = mybir.AxisListType

    def v(ap, off, dims):
        return bass_rust.AP(tensor=ap.tensor, offset=ap.offset + off, ap=dims)

    nc = tc.nc
    sh = _shapes(L)
    STRIP, TB, ROUNDS, KL, BLK = (sh["STRIP"], sh["TB"], sh["ROUNDS"],
                                  sh["KL"], sh["BLK"])
    NITER = L // BLK
    HB = BLK // 2
    G1 = min(8, KL)
    NG = KL // G1
    FLATN = (L + 1) * C

    obs_d = ins["obs"]
    trep_d = ins["trep"]
    wdesc_d = ins["wdesc"]
    tplane_d = ins["tplane"]
    len_d = ins["len_col"]
    lenm1_d = ins["lenm1"]
    einit1_d = ins["einit1"]
    path_d = outs["path"]

    vit_d = nc.dram_tensor("vit_scratch", [SEQ, FLATN], F32).ap()
    bp1_d = nc.dram_tensor("bp1_scratch", [128, STRIP * C], F32).ap()
    smap_d = nc.dram_tensor("smap_scratch", [128, C], F32).ap()
    estrip_d = nc.dram_tensor("estrip_scratch", [SEQ, NSTRIP], F32).ap()

    vec = nc.vector

    with tc.tile_pool(name="const", bufs=1) as cpool:
        trep = cpool.tile([128, C * C], F32)
        wdesc = cpool.tile([128, C], F32)
        tplane = cpool.tile([128, STRIP], F32)
        len_sb = cpool.tile([128, 1], F32)
        lenm1_sb = cpool.tile([128, 1], F32)
        nc.sync.dma_start(out=trep[:], in_=trep_d)
        nc.sync.dma_start(out=wdesc[:], in_=wdesc_d)
        nc.sync.dma_start(out=tplane[:], in_=tplane_d)
        nc.sync.dma_start(out=len_sb[:], in_=len_d)
        nc.sync.dma_start(out=lenm1_sb[:], in_=lenm1_d)

        # ============ phase 1: serial vit chain ============
        with tc.tile_pool(name="ph1", bufs=1) as pool:
            fv = pool.tile([SEQ, C], F32)
            sc = pool.tile([SEQ, C * C], F32)
            obsA = pool.tile([SEQ, HB * C], F32)
            obsB = pool.tile([SEQ, HB * C], F32)
            vitA = pool.tile([SEQ, HB * C], F32)
            vitB = pool.tile([SEQ, HB * C], F32)

            vec.memset(fv[:], 0.0)
            nc.sync.dma_start(out=v(vit_d, 0, [[FLATN, SEQ], [1, C]]), in_=fv[:])

            fvb = v(fv[:], 0, [[C, SEQ], [0, C], [1, C]])
            trep8 = v(trep[:], 0, [[C * C, SEQ], [C, C], [1, C]])
            sc3 = v(sc[:], 0, [[C * C, SEQ], [C, C], [1, C]])

            def half(obs_blk, vit_blk):
                for tau in range(HB):
                    vit_col = vit_blk[:, tau * C:(tau + 1) * C]
                    vec.tensor_tensor(out=sc[:], in0=fvb, in1=trep8, op=ALU.add)
                    vec.tensor_reduce(out=vit_col, in_=sc3, axis=AX.X, op=ALU.max)
                    vec.tensor_tensor(out=fv[:], in0=vit_col,
                                      in1=obs_blk[:, tau * C:(tau + 1) * C],
                                      op=ALU.add)

            obs_hi = v(obs_d, HB * C, [[FLATN, SEQ], [1, FLATN - HB * C]])
            vit_hi = v(vit_d, HB * C, [[FLATN, SEQ], [1, FLATN - HB * C]])
            obs_nx = v(obs_d, BLK * C, [[FLATN, SEQ], [1, FLATN - BLK * C]])
            # prologue: preload first A half
            nc.sync.dma_start(out=obsA[:], in_=obs_d[:, C:C + HB * C])
            # main loop: NITER-1 blocks, prefetching the next A half
            with tc.For_i(C, C + (NITER - 1) * BLK * C, BLK * C,
                          hint_engines=(mybir.EngineType.DVE,),
                          staggered_reset=True) as iofs:
                nc.sync.dma_start(out=obsB[:], in_=obs_hi[:, bass.ds(iofs, HB * C)])
                half(obsA, vitA)
                nc.sync.dma_start(out=obsA[:], in_=obs_nx[:, bass.ds(iofs, HB * C)])
                nc.sync.dma_start(out=vit_d[:, bass.ds(iofs, HB * C)], in_=vitA[:])
                half(obsB, vitB)
                nc.sync.dma_start(out=vit_hi[:, bass.ds(iofs, HB * C)], in_=vitB[:])
            # epilogue: last block with static offsets (obsA already prefetched)
            lo = C + (NITER - 1) * BLK * C
            nc.sync.dma_start(out=obsB[:], in_=obs_d[:, lo + HB * C:lo + BLK * C])
            half(obsA, vitA)
            nc.sync.dma_start(out=vit_d[:, lo:lo + HB * C], in_=vitA[:])
            half(obsB, vitB)
            nc.sync.dma_start(out=vit_d[:, lo + HB * C:lo + BLK * C], in_=vitB[:])

        tc.strict_bb_all_engine_barrier()

        # ============ phase 2: backpointer extraction ============
        with tc.tile_pool(name="ph2", bufs=2) as pool:
            for r in range(ROUNDS):
                off = r * TB * C
                vit_blk = pool.tile([128, (TB + 1) * C], F32, tag="vit")
                obs_blk = pool.tile([128, (TB + 1) * C], F32, tag="obs")
                fv_blk = pool.tile([128, (TB + 1) * C], F32, tag="fv")
                src_dims = [[STRIP * C, NSTRIP], [FLATN, SEQ], [1, (TB + 1) * C]]
                nc.sync.dma_start(out=vit_blk[:], in_=v(vit_d, off, src_dims))
                nc.sync.dma_start(out=obs_blk[:], in_=v(obs_d, off, src_dims))
                vec.tensor_tensor(out=fv_blk[:], in0=vit_blk[:], in1=obs_blk[:],
                                  op=ALU.add)

                P = lambda t: t[:].ap[0]
                sc2 = pool.tile([128, C * TB * C], F32, tag="sc")
                eq2 = pool.tile([128, C * TB * C], F32, tag="eq")
                vec.tensor_tensor(
                    out=sc2[:],
                    in0=v(fv_blk[:], 0, [P(fv_blk), [0, C], [C, TB], [1, C]]),
                    in1=v(trep[:], 0, [P(trep), [C, C], [0, TB], [1, C]]),
                    op=ALU.add)
                vec.tensor_tensor(
                    out=eq2[:],
                    in0=v(sc2[:], 0, [P(sc2), [TB * C, C], [C, TB], [1, C]]),
                    in1=v(vit_blk[:], C, [P(vit_blk), [1, C], [C, TB], [0, C]]),
                    op=ALU.is_equal)
                vec.tensor_tensor(
                    out=eq2[:],
                    in0=v(eq2[:], 0, [P(eq2), [TB * C, C], [C, TB], [1, C]]),
                    in1=v(wdesc[:], 0, [P(wdesc), [0, C], [0, TB], [1, C]]),
                    op=ALU.mult)
                bpw = pool.tile([128, C * TB], F32, tag="bpw")
                vec.tensor_reduce(
                    out=bpw[:],
                    in_=v(eq2[:], 0, [P(eq2), [TB * C, C], [C, TB], [1, C]]),
                    axis=AX.X, op=ALU.max)
                bp1 = pool.tile([128, C * TB], F32, tag="bp1")
                vec.tensor_scalar(out=bp1[:], in0=bpw[:], scalar1=-1.0, scalar2=9.0,
                                  op0=ALU.mult, op1=ALU.add)

                fm = pool.tile([128, TB], F32, tag="fm")
                vec.tensor_reduce(
                    out=fm[:],
                    in_=v(fv_blk[:], C, [P(fv_blk), [C, TB], [1, C]]),
                    axis=AX.X, op=ALU.max)
                eqn = pool.tile([128, TB * C], F32, tag="eqn")
                vec.tensor_tensor(
                    out=eqn[:],
                    in0=v(fv_blk[:], C, [P(fv_blk), [C, TB], [1, C]]),
                    in1=v(fm[:], 0, [P(fm), [1, TB], [0, C]]),
                    op=ALU.is_equal)
                vec.tensor_tensor(
                    out=eqn[:],
                    in0=v(eqn[:], 0, [P(eqn), [C, TB], [1, C]]),
                    in1=v(wdesc[:], 0, [P(wdesc), [0, TB], [1, C]]),
                    op=ALU.mult)
                mn = pool.tile([128, TB], F32, tag="mn")
                vec.tensor_reduce(
                    out=mn[:],
                    in_=v(eqn[:], 0, [P(eqn), [C, TB], [1, C]]),
                    axis=AX.X, op=ALU.max)
                en1 = pool.tile([128, TB], F32, tag="en1")
                vec.tensor_scalar(out=en1[:], in0=mn[:], scalar1=-1.0, scalar2=9.0,
                                  op0=ALU.mult, op1=ALU.add)
                endsel = pool.tile([128, TB], F32, tag="endsel")
                tmp = pool.tile([128, TB], F32, tag="tmpsel")
                for j in range(C):
                    dst = endsel if j == 0 else tmp
                    vec.scalar_tensor_tensor(
                        out=dst[:], in0=en1[:], scalar=float(j + 1),
                        in1=bp1[:, j * TB:(j + 1) * TB],
                        op0=ALU.is_equal, op1=ALU.mult)
                    if j > 0:
                        vec.tensor_tensor(out=endsel[:], in0=endsel[:], in1=tmp[:],
                                          op=ALU.max)
                atm = pool.tile([128, TB], F32, tag="atm")
                vec.tensor_scalar(out=atm[:], in0=tplane[:, r * TB:(r + 1) * TB],
                                  scalar1=lenm1_sb[:], scalar2=None, op0=ALU.is_equal)
                bpt1 = pool.tile([128, TB * C], F32, tag="bpt1")
                dsel = pool.tile([128, TB * C], F32, tag="dsel")
                bp1_tn = v(bp1[:], 0, [P(bp1), [1, TB], [TB, C]])
                vec.tensor_tensor(
                    out=dsel[:],
                    in0=v(endsel[:], 0, [P(endsel), [1, TB], [0, C]]),
                    in1=bp1_tn, op=ALU.subtract)
                vec.tensor_tensor(
                    out=dsel[:],
                    in0=v(dsel[:], 0, [P(dsel), [C, TB], [1, C]]),
                    in1=v(atm[:], 0, [P(atm), [1, TB], [0, C]]),
                    op=ALU.mult)
                vec.tensor_tensor(out=bpt1[:], in0=bp1_tn, in1=dsel[:], op=ALU.add)
                nc.sync.dma_start(out=bp1_d[:, off:off + TB * C], in_=bpt1[:])

        tc.strict_bb_all_engine_barrier()

        # ============ phase 3: chunked backward ============
        with tc.tile_pool(name="ph3", bufs=1) as pool:
            P = lambda t: t[:].ap[0]
            bp_strip = pool.tile([128, STRIP * C], F32)
            nc.sync.dma_start(out=bp_strip[:], in_=bp1_d[:])
            einit1 = pool.tile([128, C * KL], F32)
            nc.sync.dma_start(out=einit1[:], in_=einit1_d)
            cand1 = pool.tile([128, C * KL * S], F32)
            acc = pool.tile([128, C * KL], F32)
            tmp = pool.tile([128, C * KL], F32)

            def cand_col(tl):
                return v(cand1[:], tl, [P(cand1), [KL * S, C], [S, KL]])

            for tl in range(S - 1, -1, -1):
                if tl == S - 1:
                    prev = v(einit1[:], 0, [P(einit1), [KL, C], [1, KL]])
                else:
                    prev = cand_col(tl + 1)
                for j in range(C):
                    dst = acc[:] if j == 0 else tmp[:]
                    vec.scalar_tensor_tensor(
                        out=dst, in0=prev, scalar=float(j + 1),
                        in1=v(bp_strip[:], tl * C + j,
                              [P(bp_strip), [0, C], [S * C, KL]]),
                        op0=ALU.is_equal, op1=ALU.mult)
                    if j > 0:
                        out_ap = cand_col(tl) if j == C - 1 else acc[:]
                        vec.tensor_tensor(out=out_ap, in0=acc[:], in1=tmp[:],
                                          op=ALU.max)

            m1a = pool.tile([128, C * NG], F32)
            m1b = pool.tile([128, C * NG], F32)
            t1 = pool.tile([128, C * NG], F32)
            a1 = pool.tile([128, C * NG], F32)
            vec.tensor_copy(out=m1a[:],
                            in_=v(einit1[:], 0, [P(einit1), [KL, C], [G1, NG]]))
            cur, nxt = m1a, m1b
            for kk in range(G1 - 1, -1, -1):
                for j in range(C):
                    dst = a1[:] if j == 0 else t1[:]
                    vec.scalar_tensor_tensor(
                        out=dst, in0=cur[:], scalar=float(j + 1),
                        in1=v(cand1[:], j * KL * S + kk * S,
                              [P(cand1), [0, C], [G1 * S, NG]]),
                        op0=ALU.is_equal, op1=ALU.mult)
                    if j > 0:
                        out_ap = nxt[:] if j == C - 1 else a1[:]
                        vec.tensor_tensor(out=out_ap, in0=a1[:], in1=t1[:],
                                          op=ALU.max)
                cur, nxt = nxt, cur
            m1 = cur

            msa = pool.tile([128, C], F32)
            msb = pool.tile([128, C], F32)
            t2 = pool.tile([128, C], F32)
            a2 = pool.tile([128, C], F32)
            vec.tensor_copy(out=msa[:], in_=v(einit1[:], 0,
                                              [P(einit1), [KL, C], [1, 1]]))
            cur2, nxt2 = msa, msb
            for g in range(NG - 1, -1, -1):
                for j in range(C):
                    dst = a2[:] if j == 0 else t2[:]
                    vec.scalar_tensor_tensor(
                        out=dst, in0=cur2[:], scalar=float(j + 1),
                        in1=v(m1[:], j * NG + g, [P(m1), [0, C], [0, 1]]),
                        op0=ALU.is_equal, op1=ALU.mult)
                    if j > 0:
                        out_ap = nxt2[:] if j == C - 1 else a2[:]
                        vec.tensor_tensor(out=out_ap, in0=a2[:], in1=t2[:],
                                          op=ALU.max)
                cur2, nxt2 = nxt2, cur2
            nc.sync.dma_start(out=smap_d[:], in_=cur2[:])
            tc.strict_bb_all_engine_barrier()

            smap_t = pool.tile([SEQ, NSTRIP * C], F32)
            nc.sync.dma_start(out=smap_t[:],
                              in_=v(smap_d, 0, [[C, SEQ], [C * SEQ, NSTRIP], [1, C]]))
            state = pool.tile([SEQ, 1], F32)
            sacc = pool.tile([SEQ, 1], F32)
            stmp = pool.tile([SEQ, 1], F32)
            estrip = pool.tile([SEQ, NSTRIP], F32)
            vec.memset(state[:], 1.0)
            for sg in range(NSTRIP - 1, -1, -1):
                vec.tensor_copy(out=estrip[:, sg:sg + 1], in_=state[:])
                for j in range(C):
                    dst = sacc if j == 0 else stmp
                    vec.scalar_tensor_tensor(
                        out=dst[:], in0=state[:], scalar=float(j + 1),
                        in1=smap_t[:, sg * C + j:sg * C + j + 1],
                        op0=ALU.is_equal, op1=ALU.mult)
                    if j > 0:
                        out_ap = state[:] if j == C - 1 else sacc[:]
                        vec.tensor_tensor(out=out_ap, in0=sacc[:], in1=stmp[:],
                                          op=ALU.max)
            nc.sync.dma_start(out=estrip_d, in_=estrip[:])
            tc.strict_bb_all_engine_barrier()
            eseed = pool.tile([128, 1], F32)
            nc.sync.dma_start(out=eseed[:],
                              in_=v(estrip_d, 0, [[1, NSTRIP], [NSTRIP, SEQ], [1, 1]]))

            eg = pool.tile([128, NG], F32)
            st2 = pool.tile([128, 1], F32)
            d2a = pool.tile([128, 1], F32)
            d2t = pool.tile([128, 1], F32)
            vec.tensor_copy(out=st2[:], in_=eseed[:])
            for g in range(NG - 1, -1, -1):
                vec.tensor_copy(out=eg[:, g:g + 1], in_=st2[:])
                for j in range(C):
                    dst = d2a if j == 0 else d2t
                    vec.scalar_tensor_tensor(
                        out=dst[:], in0=st2[:], scalar=float(j + 1),
                        in1=v(m1[:], j * NG + g, [P(m1), [0, 1]]),
                        op0=ALU.is_equal, op1=ALU.mult)
                    if j > 0:
                        out_ap = st2[:] if j == C - 1 else d2a[:]
                        vec.tensor_tensor(out=out_ap, in0=d2a[:], in1=d2t[:],
                                          op=ALU.max)

            ek = pool.tile([128, KL], F32)
            st3 = pool.tile([128, NG], F32)
            d1a = pool.tile([128, NG], F32)
            d1t = pool.tile([128, NG], F32)
            vec.tensor_copy(out=st3[:], in_=eg[:])
            for kk in range(G1 - 1, -1, -1):
                vec.tensor_copy(out=v(ek[:], kk, [P(ek), [G1, NG]]), in_=st3[:])
                for j in range(C):
                    dst = d1a if j == 0 else d1t
                    vec.scalar_tensor_tensor(
                        out=dst[:], in0=st3[:], scalar=float(j + 1),
                        in1=v(cand1[:], j * KL * S + kk * S,
                              [P(cand1), [G1 * S, NG]]),
                        op0=ALU.is_equal, op1=ALU.mult)
                    if j > 0:
                        out_ap = st3[:] if j == C - 1 else d1a[:]
                        vec.tensor_tensor(out=out_ap, in0=d1a[:], in1=d1t[:],
                                          op=ALU.max)

            acc2 = pool.tile([128, STRIP], F32)
            tsel = pool.tile([128, STRIP], F32)
            for e in range(C):
                dst = acc2 if e == 0 else tsel
                vec.scalar_tensor_tensor(
                    out=dst[:],
                    in0=v(ek[:], 0, [P(ek), [1, KL], [0, S]]),
                    scalar=float(e + 1),
                    in1=v(cand1[:], e * KL * S, [P(cand1), [S, KL], [1, S]]),
                    op0=ALU.is_equal, op1=ALU.mult)
                if e > 0:
                    vec.tensor_tensor(out=acc2[:], in0=acc2[:], in1=tsel[:],
                                      op=ALU.max)
            mask = pool.tile([128, STRIP], F32)
            vec.tensor_scalar(out=mask[:], in0=tplane[:], scalar1=len_sb[:],
                              scalar2=None, op0=ALU.is_lt)
            vec.tensor_tensor(out=acc2[:], in0=acc2[:], in1=mask[:], op=ALU.mult)
            vec.tensor_scalar(out=acc2[:], in0=acc2[:], scalar1=-1.0,
                              scalar2=None, op0=ALU.add)
            path_i = pool.tile([128, STRIP], I32)
            vec.tensor_copy(out=path_i[:], in_=acc2[:])
            nc.sync.dma_start(out=path_d, in_=path_i[:])


def _build(L):
    import concourse.bacc as bacc
    import concourse.mybir as mybir
    from concourse import tile

    sh = _shapes(L)
    nc = bacc.Bacc("TRN2", target_bir_lowering=False, debug=False,
                   num_devices=NCORES)
    F32 = mybir.dt.float32
    ins_aps = {
        "obs": nc.dram_tensor("obs", [SEQ, (L + 1) * C], F32, kind="ExternalInput").ap(),
        "trep": nc.dram_tensor("trep", [128, C * C], F32, kind="ExternalInput").ap(),
        "wdesc": nc.dram_tensor("wdesc", [128, C], F32, kind="ExternalInput").ap(),
        "tplane": nc.dram_tensor("tplane", [128, sh["STRIP"]], F32, kind="ExternalInput").ap(),
        "len_col": nc.dram_tensor("len_col", [128, 1], F32, kind="ExternalInput").ap(),
        "lenm1": nc.dram_tensor("lenm1", [128, 1], F32, kind="ExternalInput").ap(),
        "einit1": nc.dram_tensor("einit1", [128, C * sh["KL"]], F32, kind="ExternalInput").ap(),
    }
    outs_aps = {"path": nc.dram_tensor("path", [128, sh["STRIP"]], mybir.dt.int32,
                                       kind="ExternalOutput").ap()}
    with tile.TileContext(nc) as tc:
        _emit(tc, ins_aps, outs_aps, L)
    nc.compile()
    return nc


def kernel(observes, transitions, lengths):
    from concourse.bass_utils import run_bass_kernel_spmd

    observes = np.asarray(observes, np.float32)
    transitions = np.asarray(transitions, np.float32)
    lengths_np = np.asarray(lengths)
    L = observes.shape[2]

    if L not in _CACHE:
        _CACHE[L] = _build(L)
    nc = _CACHE[L]

    in_maps = [
        _host_prep(observes[SEQ * c:SEQ * (c + 1)], transitions,
                   lengths_np[SEQ * c:SEQ * (c + 1)], L)
        for c in range(NCORES)
    ]
    res = run_bass_kernel_spmd(nc, in_maps, core_ids=list(range(NCORES)))
    out = np.concatenate(
        [_host_post(res.results[c]["path"], L) for c in range(NCORES)], 0)
    return out.astype(np.int32)



# revision 2
# speedup vs baseline: 1.2502x; 1.2502x over previous
"""Batched CRF Viterbi decode (N=64, C=8, L=32768) on 8 TRN2 NeuronCores.

Self-contained kernel: takes FULL unsharded inputs, shards the batch dim
across 8 cores (data-parallel), runs a Bass/Tile kernel per core, and
gathers the full [64, 32768] int32 path.

Phase 1 uses a hand-built custom DVE op (CRF_STEP_ANT): ONE instruction
per Viterbi timestep (vs 3 stock ops), computing
  fv_t[i] = max_j fl(fl(fv_{t-1}[j] + T[i,j]) + obs_t[i])
which by monotonicity of fp rounding equals the reference's
fl(max_j fl(fv+T) + obs) bit-exactly.  Phases 2/3 (backpointer
extraction + backward traversal) are bulk-parallel stock ops.
"""
import sys
import numpy as np

if '/opt/trn_rl_repo' not in sys.path:
    sys.path.insert(0, '/opt/trn_rl_repo')

N_FULL, C, L = 64, 8, 32768
SEQ = 8          # sequences per core
NSTRIP = 16      # time strips per core for phases 2/3 (partition dim 128)
S = 16           # phase-3 chunk length
NCORES = 8
TBLK = 128       # phase-1 steps per block
PAGE = 9 * C     # 72 elems per step in the OT stream

_CACHE = {}

# ---------------------------------------------------------------- custom op
OP_NAME = "CRF_STEP_ANT"


def _register_crf_op():
    from concourse.dve_uop import (
        ENABLE, AluInp, AluOp, InpSel, OutPath, OutSel, Trigger, UopConfig,
        DveOpSpec,
    )
    from concourse.dve_spec import Spec, Src0, Src1
    from concourse import dve_ops

    if OP_NAME in dve_ops._SUB_OPCODE_FOR_NAME:
        return dve_ops._CRF_STEP_OP

    def _dp_tail(u):
        for k in range(2, 8):
            u.datapath_config[k].pass_through_alu()
        return u

    def _mk_steady():
        u = UopConfig()
        u.enable_input(InpSel.SRC_0, 0)
        u.enable_input(InpSel.SRC_1, 1)
        u.datapath_config[0].enable_alu(AluOp.ADD, AluInp.PREV_ALU_OUT,
                                        AluInp.PREV_DELAY_0)
        u.datapath_config[1].enable_alu(AluOp.MAX, AluInp.CURR_ALU_OUT,
                                        AluInp.PREV_ALU_OUT)
        u.require_inp0 = ENABLE
        u.require_inp1 = ENABLE
        u.repeat_count = C - 1
        u.trigger = (Trigger.SRC_TENSOR_DONE, Trigger.COUNT, Trigger.NONE)
        u.next_uop = (0, 2, 0)
        return _dp_tail(u)

    def _mk_obs():
        u = UopConfig()
        u.enable_input(InpSel.SRC_0, 0)
        u.enable_input(InpSel.SRC_1, 1)
        u.datapath_config[0].enable_alu(AluOp.BYPASS, AluInp.PREV_ALU_OUT)
        u.datapath_config[1].enable_alu(AluOp.ADD, AluInp.CURR_ALU_OUT,
                                        AluInp.PREV_ALU_OUT)
        u.enable_output(OutSel.ALU_OUT, OutPath.WR0_LO)
        u.require_inp0 = ENABLE
        u.require_inp1 = ENABLE
        u.repeat_count = 1
        u.trigger = (Trigger.SRC_TENSOR_DONE, Trigger.COUNT, Trigger.NONE)
        u.next_uop = (0, 3, 0)
        return _dp_tail(u)

    def _mk_reset():
        u = UopConfig()
        u.enable_input(InpSel.SRC_0, 0)
        u.enable_input(InpSel.SRC_1, 1)
        u.enable_input(InpSel.MAX_NEG, 2)
        u.datapath_config[0].enable_alu(AluOp.ADD, AluInp.PREV_ALU_OUT,
                                        AluInp.PREV_DELAY_0)
        u.datapath_config[0].pass_through_delay(1)
        u.datapath_config[1].enable_alu(AluOp.MAX, AluInp.PREV_DELAY_1,
                                        AluInp.PREV_ALU_OUT)
        u.require_inp0 = ENABLE
        u.require_inp1 = ENABLE
        u.repeat_count = 1
        u.trigger = (Trigger.SRC_TENSOR_DONE, Trigger.COUNT, Trigger.NONE)
        u.next_uop = (0, 1, 0)
        return _dp_tail(u)

    class CrfStepOp:
        name = OP_NAME
        subdim = False
        spec = Spec(body=Src0 + Src1, reference=lambda in0, in1: in0 + in1)

        def __init__(self):
            self._cache = {}

        def compile(self, ver):
            if ver not in self._cache:
                s = DveOpSpec(
                    name=self.name,
                    uops=[_mk_reset(), _mk_steady(), _mk_obs(), _mk_reset()],
                    opcode=dve_ops.get_dve_sub_opcode(self.name),
                    rd1_en=True,
                )
                s.validate(ver)
                self._cache[ver] = s
            return self._cache[ver]

    op = CrfStepOp()
    row = max(dve_ops._SUB_OPCODE_FOR_NAME.values()) + 1
    assert row < 0x20
    dve_ops._SUB_OPCODE_FOR_NAME[OP_NAME] = row
    dve_ops.OPS.append(op)
    dve_ops.CUSTOM_DVE_SPECS[OP_NAME] = op.spec
    dve_ops._CRF_STEP_OP = op
    return op


# ---------------------------------------------------------------- host side
def _shapes(L):
    STRIP = L // NSTRIP
    TB = min(128, STRIP)
    return dict(STRIP=STRIP, TB=TB, ROUNDS=STRIP // TB, KL=STRIP // S)


def _host_prep(observes_core, transitions, lengths_core, L):
    sh = _shapes(L)
    STRIP, KL = sh["STRIP"], sh["KL"]
    obs_t = np.ascontiguousarray(
        np.transpose(np.asarray(observes_core, np.float32), (0, 2, 1)))
    # scatter source: [s, t, i] contiguous + 2 blocks of zero pad
    obs_sc = np.zeros((SEQ, (L + 2 * TBLK) * C), np.float32)
    obs_sc[:, :L * C] = obs_t.reshape(SEQ, L * C)
    T = np.asarray(transitions, np.float32)
    # OT image for one block: page i of step t = [T[i,0..7], 0]
    ot1 = np.zeros((SEQ, TBLK * PAGE), np.float32)
    trow = np.zeros(PAGE, np.float32)
    for i in range(C):
        trow[i * 9:i * 9 + 8] = T[i, :]
    ot1[:, :] = np.tile(trow, TBLK)[None, :]
    lens = np.asarray(lengths_core).astype(np.float32)
    p = np.arange(128)
    return {
        "obs_sc": obs_sc,
        "ot_init": ot1,
        "trep": np.tile(T.reshape(1, C * C), (128, 1)).astype(np.float32),
        "wdesc": np.tile((C - np.arange(C, dtype=np.float32)).reshape(1, C),
                         (128, 1)),
        "tplane": ((p[:, None] // SEQ) * STRIP
                   + np.arange(STRIP)[None, :]).astype(np.float32),
        "len_col": lens[p % SEQ][:, None].astype(np.float32),
        "lenm1": (lens[p % SEQ][:, None] - 1.0).astype(np.float32),
        "einit1": np.tile((np.arange(C, dtype=np.float32)[:, None] + 1.0),
                          (1, KL)).reshape(1, C * KL).repeat(128, 0)
                    .astype(np.float32),
    }


def _host_post(path_dev, L):
    STRIP = L // NSTRIP
    return path_dev.reshape(NSTRIP, SEQ, STRIP).transpose(1, 0, 2).reshape(SEQ, L)


# ---------------------------------------------------------------- device
def _emit(tc, ins, outs, L):
    import concourse.bass as bass
    import concourse.mybir as mybir
    import bass_rust
    from concourse import dve_ops

    F32 = mybir.dt.float32
    I32 = mybir.dt.int32
    ALU = mybir.AluOpType
    AX = mybir.AxisListType

    crf_op = dve_ops._CRF_STEP_OP

    def v(ap, off, dims):
        return bass_rust.AP(tensor=ap.tensor, offset=ap.offset + off, ap=dims)

    nc = tc.nc
    sh = _shapes(L)
    STRIP, TB, ROUNDS, KL = sh["STRIP"], sh["TB"], sh["ROUNDS"], sh["KL"]
    G1 = min(8, KL)
    NG = KL // G1
    FLATN = (L + 1) * C
    NBLK = L // TBLK

    obs_d = ins["obs_sc"]
    ot_init_d = ins["ot_init"]
    trep_d = ins["trep"]
    wdesc_d = ins["wdesc"]
    tplane_d = ins["tplane"]
    len_d = ins["len_col"]
    lenm1_d = ins["lenm1"]
    einit1_d = ins["einit1"]
    path_d = outs["path"]

    fv_d = nc.dram_tensor("fv_scratch", [SEQ, FLATN], F32).ap()
    smap_d = nc.dram_tensor("smap_scratch", [128, C], F32).ap()
    estrip_d = nc.dram_tensor("estrip_scratch", [SEQ, NSTRIP], F32).ap()

    vec = nc.vector

    with tc.tile_pool(name="const", bufs=1) as cpool:
        trep = cpool.tile([128, C * C], F32)
        wdesc = cpool.tile([128, C], F32)
        tplane = cpool.tile([128, STRIP], F32)
        len_sb = cpool.tile([128, 1], F32)
        lenm1_sb = cpool.tile([128, 1], F32)
        nc.sync.dma_start(out=trep[:], in_=trep_d)
        nc.sync.dma_start(out=wdesc[:], in_=wdesc_d)
        nc.sync.dma_start(out=tplane[:], in_=tplane_d)
        nc.sync.dma_start(out=len_sb[:], in_=len_d)
        nc.sync.dma_start(out=lenm1_sb[:], in_=lenm1_d)

        # ============ phase 1: fused custom-op chain ============
        with tc.tile_pool(name="ph1", bufs=1) as pool:
            otA = pool.tile([SEQ, TBLK * PAGE], F32)
            otB = pool.tile([SEQ, TBLK * PAGE], F32)
            fvA = pool.tile([SEQ, TBLK * C + C], F32)
            fvB = pool.tile([SEQ, TBLK * C + C], F32)
            vec.memset(fvA[:], 0.0)
            vec.memset(fvB[:], 0.0)
            nc.sync.dma_start(out=otA[:], in_=ot_init_d)
            nc.sync.dma_start(out=otB[:], in_=ot_init_d)
            # fv_d col 0 = fv_init = 0
            nc.sync.dma_start(out=v(fv_d, 0, [[FLATN, SEQ], [1, C]]),
                              in_=fvA[:, 0:C])

            P = lambda t: t[:].ap[0]
            SC_SB = [[PAGE, TBLK], [9, C]]     # obs slots in an OT tile

            def steps(ot_t, fv_t, fv_prev_t):
                for t in range(TBLK):
                    src = (v(fv_prev_t[:], (TBLK - 1) * C,
                             [P(fv_prev_t), [0, C], [1, 9]]) if t == 0 else
                           v(fv_t[:], (t - 1) * C, [P(fv_t), [0, C], [1, 9]]))
                    nc.vector._custom_dve(
                        crf_op,
                        out=fv_t[:, t * C:(t + 1) * C],
                        in0=v(ot_t[:], t * PAGE, [P(ot_t), [9, C], [1, 9]]),
                        in1=src)

            # prologue: scatter obs for block 0 into otA
            nc.sync.dma_start(out=v(otA[:], C, [P(otA)] + SC_SB),
                              in_=obs_d[:, 0:TBLK * C])
            BB = TBLK * C  # elems per block in obs_sc AND in fv_d cols
            OBS_ROW = (L + 2 * TBLK) * C
            obs_p1 = v(obs_d, BB, [[OBS_ROW, SEQ], [1, OBS_ROW - BB]])
            obs_p2 = v(obs_d, 2 * BB, [[OBS_ROW, SEQ], [1, OBS_ROW - 2 * BB]])
            fv_lo = v(fv_d, C, [[FLATN, SEQ], [1, FLATN - C]])
            fv_hi = v(fv_d, C + BB, [[FLATN, SEQ], [1, FLATN - C - BB]])
            with tc.For_i(0, (NBLK - 2) * BB, 2 * BB,
                          hint_engines=(mybir.EngineType.DVE,),
                          staggered_reset=True) as iofs:
                nc.sync.dma_start(out=v(otB[:], C, [P(otB)] + SC_SB),
                                  in_=obs_p1[:, bass.ds(iofs, BB)])
                steps(otA, fvA, fvB)
                nc.sync.dma_start(out=fv_lo[:, bass.ds(iofs, BB)],
                                  in_=fvA[:, 0:BB])
                nc.sync.dma_start(out=v(otA[:], C, [P(otA)] + SC_SB),
                                  in_=obs_p2[:, bass.ds(iofs, BB)])
                steps(otB, fvB, fvA)
                nc.sync.dma_start(out=fv_hi[:, bass.ds(iofs, BB)],
                                  in_=fvB[:, 0:BB])
            # epilogue: last two blocks with static offsets
            lo = (NBLK - 2) * BB
            nc.sync.dma_start(out=v(otB[:], C, [P(otB)] + SC_SB),
                              in_=obs_d[:, lo + BB:lo + 2 * BB])
            steps(otA, fvA, fvB)
            nc.sync.dma_start(out=v(fv_d, C + lo, [[FLATN, SEQ], [1, BB]]),
                              in_=fvA[:, 0:BB])
            steps(otB, fvB, fvA)
            nc.sync.dma_start(out=v(fv_d, C + lo + BB,
                                    [[FLATN, SEQ], [1, BB]]),
                              in_=fvB[:, 0:BB])

        tc.strict_bb_all_engine_barrier()

        # ============ phase 2: backpointer extraction ============
        # bp stays in SBUF end-to-end: the 8 MB DRAM round-trip costs ~1 ms
        # of sync-queue descriptor generation alone.
        bp_pool = tc.alloc_tile_pool(name="bp", bufs=1)
        bp_sb = bp_pool.tile([128, STRIP * C], F32)
        fvpool = tc.alloc_tile_pool(name="ph2fv", bufs=2)
        with tc.tile_pool(name="ph2", bufs=1) as pool:
            for r in range(ROUNDS):
                off = r * TB * C
                fv_blk = fvpool.tile([128, (TB + 1) * C], F32, tag="fv",
                                     name=f"fvblk{r}")
                src_dims = [[STRIP * C, NSTRIP], [FLATN, SEQ],
                            [1, (TB + 1) * C]]
                nc.sync.dma_start(out=fv_blk[:], in_=v(fv_d, off, src_dims))

                P = lambda t: t[:].ap[0]
                sc2 = pool.tile([128, C * TB * C], F32, tag="sc")
                eq2 = pool.tile([128, C * TB * C], F32, tag="eq")
                vitr = pool.tile([128, C * TB], F32, tag="vitr")
                vec.tensor_tensor(
                    out=sc2[:],
                    in0=v(fv_blk[:], 0, [P(fv_blk), [0, C], [C, TB], [1, C]]),
                    in1=v(trep[:], 0, [P(trep), [C, C], [0, TB], [1, C]]),
                    op=ALU.add)
                vec.tensor_reduce(
                    out=vitr[:],
                    in_=v(sc2[:], 0, [P(sc2), [TB * C, C], [C, TB], [1, C]]),
                    axis=AX.X, op=ALU.max)
                vec.tensor_tensor(
                    out=eq2[:],
                    in0=v(sc2[:], 0, [P(sc2), [TB * C, C], [C, TB], [1, C]]),
                    in1=v(vitr[:], 0, [P(vitr), [TB, C], [1, TB], [0, C]]),
                    op=ALU.is_equal)
                vec.tensor_tensor(
                    out=eq2[:],
                    in0=v(eq2[:], 0, [P(eq2), [TB * C, C], [C, TB], [1, C]]),
                    in1=v(wdesc[:], 0, [P(wdesc), [0, C], [0, TB], [1, C]]),
                    op=ALU.mult)
                bpw = pool.tile([128, C * TB], F32, tag="bpw")
                vec.tensor_reduce(
                    out=bpw[:],
                    in_=v(eq2[:], 0, [P(eq2), [TB * C, C], [C, TB], [1, C]]),
                    axis=AX.X, op=ALU.max)
                bp1 = pool.tile([128, C * TB], F32, tag="bp1")
                vec.tensor_scalar(out=bp1[:], in0=bpw[:], scalar1=-1.0,
                                  scalar2=9.0, op0=ALU.mult, op1=ALU.add)

                fm = pool.tile([128, TB], F32, tag="fm")
                vec.tensor_reduce(
                    out=fm[:],
                    in_=v(fv_blk[:], C, [P(fv_blk), [C, TB], [1, C]]),
                    axis=AX.X, op=ALU.max)
                eqn = pool.tile([128, TB * C], F32, tag="eqn")
                vec.tensor_tensor(
                    out=eqn[:],
                    in0=v(fv_blk[:], C, [P(fv_blk), [C, TB], [1, C]]),
                    in1=v(fm[:], 0, [P(fm), [1, TB], [0, C]]),
                    op=ALU.is_equal)
                vec.tensor_tensor(
                    out=eqn[:],
                    in0=v(eqn[:], 0, [P(eqn), [C, TB], [1, C]]),
                    in1=v(wdesc[:], 0, [P(wdesc), [0, TB], [1, C]]),
                    op=ALU.mult)
                mn = pool.tile([128, TB], F32, tag="mn")
                vec.tensor_reduce(
                    out=mn[:],
                    in_=v(eqn[:], 0, [P(eqn), [C, TB], [1, C]]),
                    axis=AX.X, op=ALU.max)
                en1 = pool.tile([128, TB], F32, tag="en1")
                vec.tensor_scalar(out=en1[:], in0=mn[:], scalar1=-1.0,
                                  scalar2=9.0, op0=ALU.mult, op1=ALU.add)
                endsel = pool.tile([128, TB], F32, tag="endsel")
                tmp = pool.tile([128, TB], F32, tag="tmpsel")
                for j in range(C):
                    dst = endsel if j == 0 else tmp
                    vec.scalar_tensor_tensor(
                        out=dst[:], in0=en1[:], scalar=float(j + 1),
                        in1=bp1[:, j * TB:(j + 1) * TB],
                        op0=ALU.is_equal, op1=ALU.mult)
                    if j > 0:
                        vec.tensor_tensor(out=endsel[:], in0=endsel[:],
                                          in1=tmp[:], op=ALU.max)
                atm = pool.tile([128, TB], F32, tag="atm")
                vec.tensor_scalar(out=atm[:], in0=tplane[:, r * TB:(r + 1) * TB],
                                  scalar1=lenm1_sb[:], scalar2=None,
                                  op0=ALU.is_equal)
                dsel = pool.tile([128, TB * C], F32, tag="dsel")
                bp1_tn = v(bp1[:], 0, [P(bp1), [1, TB], [TB, C]])
                vec.tensor_tensor(
                    out=dsel[:],
                    in0=v(endsel[:], 0, [P(endsel), [1, TB], [0, C]]),
                    in1=bp1_tn, op=ALU.subtract)
                vec.tensor_tensor(
                    out=dsel[:],
                    in0=v(dsel[:], 0, [P(dsel), [C, TB], [1, C]]),
                    in1=v(atm[:], 0, [P(atm), [1, TB], [0, C]]),
                    op=ALU.mult)
                vec.tensor_tensor(out=bp_sb[:, off:off + TB * C],
                                  in0=bp1_tn, in1=dsel[:], op=ALU.add)

        fvpool.release()
        tc.strict_bb_all_engine_barrier()

        # ============ phase 3: chunked backward ============
        with tc.tile_pool(name="ph3", bufs=1) as pool:
            P = lambda t: t[:].ap[0]
            bp_strip = bp_sb
            einit1 = pool.tile([128, C * KL], F32)
            nc.sync.dma_start(out=einit1[:], in_=einit1_d)
            cand1 = pool.tile([128, C * KL * S], F32)
            acc = pool.tile([128, C * KL], F32)
            tmp = pool.tile([128, C * KL], F32)

            def cand_col(tl):
                return v(cand1[:], tl, [P(cand1), [KL * S, C], [S, KL]])

            for tl in range(S - 1, -1, -1):
                if tl == S - 1:
                    prev = v(einit1[:], 0, [P(einit1), [KL, C], [1, KL]])
                else:
                    prev = cand_col(tl + 1)
                for j in range(C):
                    dst = acc[:] if j == 0 else tmp[:]
                    vec.scalar_tensor_tensor(
                        out=dst, in0=prev, scalar=float(j + 1),
                        in1=v(bp_strip[:], tl * C + j,
                              [P(bp_strip), [0, C], [S * C, KL]]),
                        op0=ALU.is_equal, op1=ALU.mult)
                    if j > 0:
                        out_ap = cand_col(tl) if j == C - 1 else acc[:]
                        vec.tensor_tensor(out=out_ap, in0=acc[:], in1=tmp[:],
                                          op=ALU.max)

            m1a = pool.tile([128, C * NG], F32)
            m1b = pool.tile([128, C * NG], F32)
            t1 = pool.tile([128, C * NG], F32)
            a1 = pool.tile([128, C * NG], F32)
            vec.tensor_copy(out=m1a[:],
                            in_=v(einit1[:], 0, [P(einit1), [KL, C], [G1, NG]]))
            cur, nxt = m1a, m1b
            for kk in range(G1 - 1, -1, -1):
                for j in range(C):
                    dst = a1[:] if j == 0 else t1[:]
                    vec.scalar_tensor_tensor(
                        out=dst, in0=cur[:], scalar=float(j + 1),
                        in1=v(cand1[:], j * KL * S + kk * S,
                              [P(cand1), [0, C], [G1 * S, NG]]),
                        op0=ALU.is_equal, op1=ALU.mult)
                    if j > 0:
                        out_ap = nxt[:] if j == C - 1 else a1[:]
                        vec.tensor_tensor(out=out_ap, in0=a1[:], in1=t1[:],
                                          op=ALU.max)
                cur, nxt = nxt, cur
            m1 = cur

            msa = pool.tile([128, C], F32)
            msb = pool.tile([128, C], F32)
            t2 = pool.tile([128, C], F32)
            a2 = pool.tile([128, C], F32)
            vec.tensor_copy(out=msa[:], in_=v(einit1[:], 0,
                                              [P(einit1), [KL, C], [1, 1]]))
            cur2, nxt2 = msa, msb
            for g in range(NG - 1, -1, -1):
                for j in range(C):
                    dst = a2[:] if j == 0 else t2[:]
                    vec.scalar_tensor_tensor(
                        out=dst, in0=cur2[:], scalar=float(j + 1),
                        in1=v(m1[:], j * NG + g, [P(m1), [0, C], [0, 1]]),
                        op0=ALU.is_equal, op1=ALU.mult)
                    if j > 0:
                        out_ap = nxt2[:] if j == C - 1 else a2[:]
                        vec.tensor_tensor(out=out_ap, in0=a2[:], in1=t2[:],
                                          op=ALU.max)
                cur2, nxt2 = nxt2, cur2
            nc.sync.dma_start(out=smap_d[:], in_=cur2[:])
            tc.strict_bb_all_engine_barrier()

            smap_t = pool.tile([SEQ, NSTRIP * C], F32)
            nc.sync.dma_start(out=smap_t[:],
                              in_=v(smap_d, 0, [[C, SEQ], [C * SEQ, NSTRIP],
                                                [1, C]]))
            state = pool.tile([SEQ, 1], F32)
            sacc = pool.tile([SEQ, 1], F32)
            stmp = pool.tile([SEQ, 1], F32)
            estrip = pool.tile([SEQ, NSTRIP], F32)
            vec.memset(state[:], 1.0)
            for sg in range(NSTRIP - 1, -1, -1):
                vec.tensor_copy(out=estrip[:, sg:sg + 1], in_=state[:])
                for j in range(C):
                    dst = sacc if j == 0 else stmp
                    vec.scalar_tensor_tensor(
                        out=dst[:], in0=state[:], scalar=float(j + 1),
                        in1=smap_t[:, sg * C + j:sg * C + j + 1],
                        op0=ALU.is_equal, op1=ALU.mult)
                    if j > 0:
                        out_ap = state[:] if j == C - 1 else sacc[:]
                        vec.tensor_tensor(out=out_ap, in0=sacc[:], in1=stmp[:],
                                          op=ALU.max)
            nc.sync.dma_start(out=estrip_d, in_=estrip[:])
            tc.strict_bb_all_engine_barrier()
            eseed = pool.tile([128, 1], F32)
            nc.sync.dma_start(out=eseed[:],
                              in_=v(estrip_d, 0, [[1, NSTRIP], [NSTRIP, SEQ],
                                                  [1, 1]]))

            eg = pool.tile([128, NG], F32)
            st2 = pool.tile([128, 1], F32)
            d2a = pool.tile([128, 1], F32)
            d2t = pool.tile([128, 1], F32)
            vec.tensor_copy(out=st2[:], in_=eseed[:])
            for g in range(NG - 1, -1, -1):
                vec.tensor_copy(out=eg[:, g:g + 1], in_=st2[:])
                for j in range(C):
                    dst = d2a if j == 0 else d2t
                    vec.scalar_tensor_tensor(
                        out=dst[:], in0=st2[:], scalar=float(j + 1),
                        in1=v(m1[:], j * NG + g, [P(m1), [0, 1]]),
                        op0=ALU.is_equal, op1=ALU.mult)
                    if j > 0:
                        out_ap = st2[:] if j == C - 1 else d2a[:]
                        vec.tensor_tensor(out=out_ap, in0=d2a[:], in1=d2t[:],
                                          op=ALU.max)

            ek = pool.tile([128, KL], F32)
            st3 = pool.tile([128, NG], F32)
            d1a = pool.tile([128, NG], F32)
            d1t = pool.tile([128, NG], F32)
            vec.tensor_copy(out=st3[:], in_=eg[:])
            for kk in range(G1 - 1, -1, -1):
                vec.tensor_copy(out=v(ek[:], kk, [P(ek), [G1, NG]]), in_=st3[:])
                for j in range(C):
                    dst = d1a if j == 0 else d1t
                    vec.scalar_tensor_tensor(
                        out=dst[:], in0=st3[:], scalar=float(j + 1),
                        in1=v(cand1[:], j * KL * S + kk * S,
                              [P(cand1), [G1 * S, NG]]),
                        op0=ALU.is_equal, op1=ALU.mult)
                    if j > 0:
                        out_ap = st3[:] if j == C - 1 else d1a[:]
                        vec.tensor_tensor(out=out_ap, in0=d1a[:], in1=d1t[:],
                                          op=ALU.max)

            acc2 = pool.tile([128, STRIP], F32)
            tsel = pool.tile([128, STRIP], F32)
            for e in range(C):
                dst = acc2 if e == 0 else tsel
                vec.scalar_tensor_tensor(
                    out=dst[:],
                    in0=v(ek[:], 0, [P(ek), [1, KL], [0, S]]),
                    scalar=float(e + 1),
                    in1=v(cand1[:], e * KL * S, [P(cand1), [S, KL], [1, S]]),
                    op0=ALU.is_equal, op1=ALU.mult)
                if e > 0:
                    vec.tensor_tensor(out=acc2[:], in0=acc2[:], in1=tsel[:],
                                      op=ALU.max)
            mask = pool.tile([128, STRIP], F32)
            vec.tensor_scalar(out=mask[:], in0=tplane[:], scalar1=len_sb[:],
                              scalar2=None, op0=ALU.is_lt)
            vec.tensor_tensor(out=acc2[:], in0=acc2[:], in1=mask[:],
                              op=ALU.mult)
            vec.tensor_scalar(out=acc2[:], in0=acc2[:], scalar1=-1.0,
                              scalar2=None, op0=ALU.add)
            path_i = pool.tile([128, STRIP], I32)
            vec.tensor_copy(out=path_i[:], in_=acc2[:])
            nc.sync.dma_start(out=path_d, in_=path_i[:])
        bp_pool.release()


def _build(L):
    import concourse.bacc as bacc
    import concourse.mybir as mybir
    from concourse import tile

    _register_crf_op()
    sh = _shapes(L)
    nc = bacc.Bacc("TRN2", target_bir_lowering=False, debug=False,
                   num_devices=NCORES)
    F32 = mybir.dt.float32
    ins_aps = {
        "obs_sc": nc.dram_tensor("obs_sc", [SEQ, (L + 2 * TBLK) * C], F32,
                                 kind="ExternalInput").ap(),
        "ot_init": nc.dram_tensor("ot_init", [SEQ, TBLK * PAGE], F32,
                                  kind="ExternalInput").ap(),
        "trep": nc.dram_tensor("trep", [128, C * C], F32,
                               kind="ExternalInput").ap(),
        "wdesc": nc.dram_tensor("wdesc", [128, C], F32,
                                kind="ExternalInput").ap(),
        "tplane": nc.dram_tensor("tplane", [128, sh["STRIP"]], F32,
                                 kind="ExternalInput").ap(),
        "len_col": nc.dram_tensor("len_col", [128, 1], F32,
                                  kind="ExternalInput").ap(),
        "lenm1": nc.dram_tensor("lenm1", [128, 1], F32,
                                kind="ExternalInput").ap(),
        "einit1": nc.dram_tensor("einit1", [128, C * sh["KL"]], F32,
                                 kind="ExternalInput").ap(),
    }
    outs_aps = {"path": nc.dram_tensor("path", [128, sh["STRIP"]],
                                       mybir.dt.int32,
                                       kind="ExternalOutput").ap()}
    with tile.TileContext(nc) as tc:
        _emit(tc, ins_aps, outs_aps, L)
    nc.compile()
    return nc


def kernel(observes, transitions, lengths):
    from concourse.bass_utils import run_bass_kernel_spmd

    observes = np.asarray(observes, np.float32)
    transitions = np.asarray(transitions, np.float32)
    lengths_np = np.asarray(lengths)
    L = observes.shape[2]

    if L not in _CACHE:
        _CACHE[L] = _build(L)
    nc = _CACHE[L]

    in_maps = [
        _host_prep(observes[SEQ * c:SEQ * (c + 1)], transitions,
                   lengths_np[SEQ * c:SEQ * (c + 1)], L)
        for c in range(NCORES)
    ]
    res = run_bass_kernel_spmd(nc, in_maps, core_ids=list(range(NCORES)))
    out = np.concatenate(
        [_host_post(res.results[c]["path"], L) for c in range(NCORES)], 0)
    return out.astype(np.int32)


# revision 3
# speedup vs baseline: 1.2835x; 1.0267x over previous
"""Batched CRF Viterbi decode (N=64, C=8, L=32768) on 8 TRN2 NeuronCores.

Self-contained kernel: takes FULL unsharded inputs, shards the batch dim
across 8 cores (data-parallel), runs a Bass/Tile kernel per core, and
gathers the full [64, 32768] int32 path.

Phase 1 uses a hand-built custom DVE op (CRF_STEP_ANT): ONE instruction
per Viterbi timestep (vs 3 stock ops), computing
  fv_t[i] = max_j fl(fl(fv_{t-1}[j] + T[i,j]) + obs_t[i])
which by monotonicity of fp rounding equals the reference's
fl(max_j fl(fv+T) + obs) bit-exactly.  Phases 2/3 (backpointer
extraction + backward traversal) are bulk-parallel stock ops.
"""
import sys
import numpy as np

if '/opt/trn_rl_repo' not in sys.path:
    sys.path.insert(0, '/opt/trn_rl_repo')

N_FULL, C, L = 64, 8, 32768
SEQ = 8          # sequences per core
NSTRIP = 16      # time strips per core for phases 2/3 (partition dim 128)
S = 16           # phase-3 chunk length
NCORES = 8
TBLK = 256       # phase-1 steps per block
PAGE = 9 * C     # 72 elems per step in the OT stream

_CACHE = {}

# ---------------------------------------------------------------- custom op
OP_NAME = "CRF_STEP_ANT"


def _register_crf_op():
    from concourse.dve_uop import (
        ENABLE, AluInp, AluOp, InpSel, OutPath, OutSel, Trigger, UopConfig,
        DveOpSpec,
    )
    from concourse.dve_spec import Spec, Src0, Src1
    from concourse import dve_ops

    if OP_NAME in dve_ops._SUB_OPCODE_FOR_NAME:
        return dve_ops._CRF_STEP_OP

    def _dp_tail(u):
        for k in range(2, 8):
            u.datapath_config[k].pass_through_alu()
        return u

    def _mk_steady():
        u = UopConfig()
        u.enable_input(InpSel.SRC_0, 0)
        u.enable_input(InpSel.SRC_1, 1)
        u.datapath_config[0].enable_alu(AluOp.ADD, AluInp.PREV_ALU_OUT,
                                        AluInp.PREV_DELAY_0)
        u.datapath_config[1].enable_alu(AluOp.MAX, AluInp.CURR_ALU_OUT,
                                        AluInp.PREV_ALU_OUT)
        u.require_inp0 = ENABLE
        u.require_inp1 = ENABLE
        u.repeat_count = C - 1
        u.trigger = (Trigger.SRC_TENSOR_DONE, Trigger.COUNT, Trigger.NONE)
        u.next_uop = (0, 2, 0)
        return _dp_tail(u)

    def _mk_obs():
        u = UopConfig()
        u.enable_input(InpSel.SRC_0, 0)
        u.enable_input(InpSel.SRC_1, 1)
        u.datapath_config[0].enable_alu(AluOp.BYPASS, AluInp.PREV_ALU_OUT)
        u.datapath_config[1].enable_alu(AluOp.ADD, AluInp.CURR_ALU_OUT,
                                        AluInp.PREV_ALU_OUT)
        u.enable_output(OutSel.ALU_OUT, OutPath.WR0_LO)
        u.require_inp0 = ENABLE
        u.require_inp1 = ENABLE
        u.repeat_count = 1
        u.trigger = (Trigger.SRC_TENSOR_DONE, Trigger.COUNT, Trigger.NONE)
        u.next_uop = (0, 3, 0)
        return _dp_tail(u)

    def _mk_reset():
        u = UopConfig()
        u.enable_input(InpSel.SRC_0, 0)
        u.enable_input(InpSel.SRC_1, 1)
        u.enable_input(InpSel.MAX_NEG, 2)
        u.datapath_config[0].enable_alu(AluOp.ADD, AluInp.PREV_ALU_OUT,
                                        AluInp.PREV_DELAY_0)
        u.datapath_config[0].pass_through_delay(1)
        u.datapath_config[1].enable_alu(AluOp.MAX, AluInp.PREV_DELAY_1,
                                        AluInp.PREV_ALU_OUT)
        u.require_inp0 = ENABLE
        u.require_inp1 = ENABLE
        u.repeat_count = 1
        u.trigger = (Trigger.SRC_TENSOR_DONE, Trigger.COUNT, Trigger.NONE)
        u.next_uop = (0, 1, 0)
        return _dp_tail(u)

    class CrfStepOp:
        name = OP_NAME
        subdim = False
        spec = Spec(body=Src0 + Src1, reference=lambda in0, in1: in0 + in1)

        def __init__(self):
            self._cache = {}

        def compile(self, ver):
            if ver not in self._cache:
                s = DveOpSpec(
                    name=self.name,
                    uops=[_mk_reset(), _mk_steady(), _mk_obs(), _mk_reset()],
                    opcode=dve_ops.get_dve_sub_opcode(self.name),
                    rd1_en=True,
                )
                s.validate(ver)
                self._cache[ver] = s
            return self._cache[ver]

    op = CrfStepOp()
    row = max(dve_ops._SUB_OPCODE_FOR_NAME.values()) + 1
    assert row < 0x20
    dve_ops._SUB_OPCODE_FOR_NAME[OP_NAME] = row
    dve_ops.OPS.append(op)
    dve_ops.CUSTOM_DVE_SPECS[OP_NAME] = op.spec
    dve_ops._CRF_STEP_OP = op
    return op


# ---------------------------------------------------------------- host side
def _shapes(L):
    STRIP = L // NSTRIP
    TB = min(128, STRIP)
    return dict(STRIP=STRIP, TB=TB, ROUNDS=STRIP // TB, KL=STRIP // S)


def _host_prep(observes_core, transitions, lengths_core, L):
    sh = _shapes(L)
    STRIP, KL = sh["STRIP"], sh["KL"]
    obs_t = np.ascontiguousarray(
        np.transpose(np.asarray(observes_core, np.float32), (0, 2, 1)))
    # scatter source: [s, t, i] contiguous + 2 blocks of zero pad
    obs_sc = np.zeros((SEQ, (L + 2 * TBLK) * C), np.float32)
    obs_sc[:, :L * C] = obs_t.reshape(SEQ, L * C)
    T = np.asarray(transitions, np.float32)
    # OT image for one block: page i of step t = [T[i,0..7], 0]
    ot1 = np.zeros((SEQ, TBLK * PAGE), np.float32)
    trow = np.zeros(PAGE, np.float32)
    for i in range(C):
        trow[i * 9:i * 9 + 8] = T[i, :]
    ot1[:, :] = np.tile(trow, TBLK)[None, :]
    lens = np.asarray(lengths_core).astype(np.float32)
    p = np.arange(128)
    return {
        "obs_sc": obs_sc,
        "ot_init": ot1,
        "trep": np.tile(T.reshape(1, C * C), (128, 1)).astype(np.float32),
        "wdesc": np.tile((C - np.arange(C, dtype=np.float32)).reshape(1, C),
                         (128, 1)),
        "tplane": ((p[:, None] // SEQ) * STRIP
                   + np.arange(STRIP)[None, :]).astype(np.float32),
        "len_col": lens[p % SEQ][:, None].astype(np.float32),
        "lenm1": (lens[p % SEQ][:, None] - 1.0).astype(np.float32),
        "einit1": np.tile((np.arange(C, dtype=np.float32)[:, None] + 1.0),
                          (1, KL)).reshape(1, C * KL).repeat(128, 0)
                    .astype(np.float32),
    }


def _host_post(path_dev, L):
    STRIP = L // NSTRIP
    return path_dev.reshape(NSTRIP, SEQ, STRIP).transpose(1, 0, 2).reshape(SEQ, L)


# ---------------------------------------------------------------- device
def _emit(tc, ins, outs, L):
    import concourse.bass as bass
    import concourse.mybir as mybir
    import bass_rust
    from concourse import dve_ops

    F32 = mybir.dt.float32
    I32 = mybir.dt.int32
    ALU = mybir.AluOpType
    AX = mybir.AxisListType

    crf_op = dve_ops._CRF_STEP_OP

    def v(ap, off, dims):
        return bass_rust.AP(tensor=ap.tensor, offset=ap.offset + off, ap=dims)

    nc = tc.nc
    sh = _shapes(L)
    STRIP, TB, ROUNDS, KL = sh["STRIP"], sh["TB"], sh["ROUNDS"], sh["KL"]
    G1 = min(8, KL)
    NG = KL // G1
    FLATN = (L + 1) * C
    NBLK = L // TBLK

    obs_d = ins["obs_sc"]
    ot_init_d = ins["ot_init"]
    trep_d = ins["trep"]
    wdesc_d = ins["wdesc"]
    tplane_d = ins["tplane"]
    len_d = ins["len_col"]
    lenm1_d = ins["lenm1"]
    einit1_d = ins["einit1"]
    path_d = outs["path"]

    fv_d = nc.dram_tensor("fv_scratch", [SEQ, FLATN], F32).ap()
    smap_d = nc.dram_tensor("smap_scratch", [128, C], F32).ap()
    estrip_d = nc.dram_tensor("estrip_scratch", [SEQ, NSTRIP], F32).ap()

    vec = nc.vector

    with tc.tile_pool(name="const", bufs=1) as cpool:
        trep = cpool.tile([128, C * C], F32)
        wdesc = cpool.tile([128, C], F32)
        tplane = cpool.tile([128, STRIP], F32)
        len_sb = cpool.tile([128, 1], F32)
        lenm1_sb = cpool.tile([128, 1], F32)
        nc.sync.dma_start(out=trep[:], in_=trep_d)
        nc.sync.dma_start(out=wdesc[:], in_=wdesc_d)
        nc.sync.dma_start(out=tplane[:], in_=tplane_d)
        nc.sync.dma_start(out=len_sb[:], in_=len_d)
        nc.sync.dma_start(out=lenm1_sb[:], in_=lenm1_d)

        # ============ phase 1: fused custom-op chain ============
        with tc.tile_pool(name="ph1", bufs=1) as pool:
            otA = pool.tile([SEQ, TBLK * PAGE], F32)
            otB = pool.tile([SEQ, TBLK * PAGE], F32)
            fvA = pool.tile([SEQ, TBLK * C + C], F32)
            fvB = pool.tile([SEQ, TBLK * C + C], F32)
            vec.memset(fvA[:], 0.0)
            vec.memset(fvB[:], 0.0)
            nc.sync.dma_start(out=otA[:], in_=ot_init_d)
            nc.sync.dma_start(out=otB[:], in_=ot_init_d)
            # fv_d col 0 = fv_init = 0
            nc.sync.dma_start(out=v(fv_d, 0, [[FLATN, SEQ], [1, C]]),
                              in_=fvA[:, 0:C])

            P = lambda t: t[:].ap[0]
            SC_SB = [[PAGE, TBLK], [9, C]]     # obs slots in an OT tile

            def steps(ot_t, fv_t, fv_prev_t):
                for t in range(TBLK):
                    src = (v(fv_prev_t[:], (TBLK - 1) * C,
                             [P(fv_prev_t), [0, C], [1, 9]]) if t == 0 else
                           v(fv_t[:], (t - 1) * C, [P(fv_t), [0, C], [1, 9]]))
                    nc.vector._custom_dve(
                        crf_op,
                        out=fv_t[:, t * C:(t + 1) * C],
                        in0=v(ot_t[:], t * PAGE, [P(ot_t), [9, C], [1, 9]]),
                        in1=src)

            # prologue: scatter obs for block 0 into otA
            nc.sync.dma_start(out=v(otA[:], C, [P(otA)] + SC_SB),
                              in_=obs_d[:, 0:TBLK * C])
            BB = TBLK * C  # elems per block in obs_sc AND in fv_d cols
            OBS_ROW = (L + 2 * TBLK) * C
            obs_p1 = v(obs_d, BB, [[OBS_ROW, SEQ], [1, OBS_ROW - BB]])
            obs_p2 = v(obs_d, 2 * BB, [[OBS_ROW, SEQ], [1, OBS_ROW - 2 * BB]])
            fv_lo = v(fv_d, C, [[FLATN, SEQ], [1, FLATN - C]])
            fv_hi = v(fv_d, C + BB, [[FLATN, SEQ], [1, FLATN - C - BB]])
            with tc.For_i(0, (NBLK - 2) * BB, 2 * BB,
                          hint_engines=(mybir.EngineType.DVE,),
                          staggered_reset=True) as iofs:
                nc.sync.dma_start(out=v(otB[:], C, [P(otB)] + SC_SB),
                                  in_=obs_p1[:, bass.ds(iofs, BB)])
                steps(otA, fvA, fvB)
                nc.sync.dma_start(out=fv_lo[:, bass.ds(iofs, BB)],
                                  in_=fvA[:, 0:BB])
                nc.sync.dma_start(out=v(otA[:], C, [P(otA)] + SC_SB),
                                  in_=obs_p2[:, bass.ds(iofs, BB)])
                steps(otB, fvB, fvA)
                nc.sync.dma_start(out=fv_hi[:, bass.ds(iofs, BB)],
                                  in_=fvB[:, 0:BB])
            # epilogue: last two blocks with static offsets
            lo = (NBLK - 2) * BB
            nc.sync.dma_start(out=v(otB[:], C, [P(otB)] + SC_SB),
                              in_=obs_d[:, lo + BB:lo + 2 * BB])
            steps(otA, fvA, fvB)
            nc.sync.dma_start(out=v(fv_d, C + lo, [[FLATN, SEQ], [1, BB]]),
                              in_=fvA[:, 0:BB])
            steps(otB, fvB, fvA)
            nc.sync.dma_start(out=v(fv_d, C + lo + BB,
                                    [[FLATN, SEQ], [1, BB]]),
                              in_=fvB[:, 0:BB])

        tc.strict_bb_all_engine_barrier()

        # ============ phase 2: backpointer extraction ============
        # bp stays in SBUF end-to-end: the 8 MB DRAM round-trip costs ~1 ms
        # of sync-queue descriptor generation alone.
        bp_pool = tc.alloc_tile_pool(name="bp", bufs=1)
        bp_sb = bp_pool.tile([128, STRIP * C], F32)
        fvpool = tc.alloc_tile_pool(name="ph2fv", bufs=2)
        with tc.tile_pool(name="ph2", bufs=1) as pool:
            for r in range(ROUNDS):
                off = r * TB * C
                fv_blk = fvpool.tile([128, (TB + 1) * C], F32, tag="fv",
                                     name=f"fvblk{r}")
                src_dims = [[STRIP * C, NSTRIP], [FLATN, SEQ],
                            [1, (TB + 1) * C]]
                nc.sync.dma_start(out=fv_blk[:], in_=v(fv_d, off, src_dims))

                P = lambda t: t[:].ap[0]
                sc2 = pool.tile([128, C * TB * C], F32, tag="sc")
                eq2 = pool.tile([128, C * TB * C], F32, tag="eq")
                vitr = pool.tile([128, C * TB], F32, tag="vitr")
                vec.tensor_tensor(
                    out=sc2[:],
                    in0=v(fv_blk[:], 0, [P(fv_blk), [0, C], [C, TB], [1, C]]),
                    in1=v(trep[:], 0, [P(trep), [C, C], [0, TB], [1, C]]),
                    op=ALU.add)
                vec.tensor_reduce(
                    out=vitr[:],
                    in_=v(sc2[:], 0, [P(sc2), [TB * C, C], [C, TB], [1, C]]),
                    axis=AX.X, op=ALU.max)
                vec.tensor_tensor(
                    out=eq2[:],
                    in0=v(sc2[:], 0, [P(sc2), [TB * C, C], [C, TB], [1, C]]),
                    in1=v(vitr[:], 0, [P(vitr), [TB, C], [1, TB], [0, C]]),
                    op=ALU.is_equal)
                vec.tensor_tensor(
                    out=eq2[:],
                    in0=v(eq2[:], 0, [P(eq2), [TB * C, C], [C, TB], [1, C]]),
                    in1=v(wdesc[:], 0, [P(wdesc), [0, C], [0, TB], [1, C]]),
                    op=ALU.mult)
                bpw = pool.tile([128, C * TB], F32, tag="bpw")
                vec.tensor_reduce(
                    out=bpw[:],
                    in_=v(eq2[:], 0, [P(eq2), [TB * C, C], [C, TB], [1, C]]),
                    axis=AX.X, op=ALU.max)
                bp1 = pool.tile([128, C * TB], F32, tag="bp1")
                vec.tensor_scalar(out=bp1[:], in0=bpw[:], scalar1=-1.0,
                                  scalar2=9.0, op0=ALU.mult, op1=ALU.add)

                fm = pool.tile([128, TB], F32, tag="fm")
                vec.tensor_reduce(
                    out=fm[:],
                    in_=v(fv_blk[:], C, [P(fv_blk), [C, TB], [1, C]]),
                    axis=AX.X, op=ALU.max)
                eqn = pool.tile([128, TB * C], F32, tag="eqn")
                vec.tensor_tensor(
                    out=eqn[:],
                    in0=v(fv_blk[:], C, [P(fv_blk), [C, TB], [1, C]]),
                    in1=v(fm[:], 0, [P(fm), [1, TB], [0, C]]),
                    op=ALU.is_equal)
                vec.tensor_tensor(
                    out=eqn[:],
                    in0=v(eqn[:], 0, [P(eqn), [C, TB], [1, C]]),
                    in1=v(wdesc[:], 0, [P(wdesc), [0, TB], [1, C]]),
                    op=ALU.mult)
                mn = pool.tile([128, TB], F32, tag="mn")
                vec.tensor_reduce(
                    out=mn[:],
                    in_=v(eqn[:], 0, [P(eqn), [C, TB], [1, C]]),
                    axis=AX.X, op=ALU.max)
                en1 = pool.tile([128, TB], F32, tag="en1")
                vec.tensor_scalar(out=en1[:], in0=mn[:], scalar1=-1.0,
                                  scalar2=9.0, op0=ALU.mult, op1=ALU.add)
                endsel = pool.tile([128, TB], F32, tag="endsel")
                tmp = pool.tile([128, TB], F32, tag="tmpsel")
                for j in range(C):
                    dst = endsel if j == 0 else tmp
                    vec.scalar_tensor_tensor(
                        out=dst[:], in0=en1[:], scalar=float(j + 1),
                        in1=bp1[:, j * TB:(j + 1) * TB],
                        op0=ALU.is_equal, op1=ALU.mult)
                    if j > 0:
                        vec.tensor_tensor(out=endsel[:], in0=endsel[:],
                                          in1=tmp[:], op=ALU.max)
                atm = pool.tile([128, TB], F32, tag="atm")
                vec.tensor_scalar(out=atm[:], in0=tplane[:, r * TB:(r + 1) * TB],
                                  scalar1=lenm1_sb[:], scalar2=None,
                                  op0=ALU.is_equal)
                dsel = pool.tile([128, TB * C], F32, tag="dsel")
                bp1_tn = v(bp1[:], 0, [P(bp1), [1, TB], [TB, C]])
                vec.tensor_tensor(
                    out=dsel[:],
                    in0=v(endsel[:], 0, [P(endsel), [1, TB], [0, C]]),
                    in1=bp1_tn, op=ALU.subtract)
                vec.tensor_tensor(
                    out=dsel[:],
                    in0=v(dsel[:], 0, [P(dsel), [C, TB], [1, C]]),
                    in1=v(atm[:], 0, [P(atm), [1, TB], [0, C]]),
                    op=ALU.mult)
                vec.tensor_tensor(out=bp_sb[:, off:off + TB * C],
                                  in0=bp1_tn, in1=dsel[:], op=ALU.add)

        fvpool.release()
        tc.strict_bb_all_engine_barrier()

        # ============ phase 3: chunked backward ============
        with tc.tile_pool(name="ph3", bufs=1) as pool:
            P = lambda t: t[:].ap[0]
            bp_strip = bp_sb
            einit1 = pool.tile([128, C * KL], F32)
            nc.sync.dma_start(out=einit1[:], in_=einit1_d)
            cand1 = pool.tile([128, C * KL * S], F32)
            acc = pool.tile([128, C * KL], F32)
            tmp = pool.tile([128, C * KL], F32)

            def cand_col(tl):
                return v(cand1[:], tl, [P(cand1), [KL * S, C], [S, KL]])

            for tl in range(S - 1, -1, -1):
                if tl == S - 1:
                    prev = v(einit1[:], 0, [P(einit1), [KL, C], [1, KL]])
                else:
                    prev = cand_col(tl + 1)
                for j in range(C):
                    dst = acc[:] if j == 0 else tmp[:]
                    vec.scalar_tensor_tensor(
                        out=dst, in0=prev, scalar=float(j + 1),
                        in1=v(bp_strip[:], tl * C + j,
                              [P(bp_strip), [0, C], [S * C, KL]]),
                        op0=ALU.is_equal, op1=ALU.mult)
                    if j > 0:
                        out_ap = cand_col(tl) if j == C - 1 else acc[:]
                        vec.tensor_tensor(out=out_ap, in0=acc[:], in1=tmp[:],
                                          op=ALU.max)

            m1a = pool.tile([128, C * NG], F32)
            m1b = pool.tile([128, C * NG], F32)
            t1 = pool.tile([128, C * NG], F32)
            a1 = pool.tile([128, C * NG], F32)
            vec.tensor_copy(out=m1a[:],
                            in_=v(einit1[:], 0, [P(einit1), [KL, C], [G1, NG]]))
            cur, nxt = m1a, m1b
            for kk in range(G1 - 1, -1, -1):
                for j in range(C):
                    dst = a1[:] if j == 0 else t1[:]
                    vec.scalar_tensor_tensor(
                        out=dst, in0=cur[:], scalar=float(j + 1),
                        in1=v(cand1[:], j * KL * S + kk * S,
                              [P(cand1), [0, C], [G1 * S, NG]]),
                        op0=ALU.is_equal, op1=ALU.mult)
                    if j > 0:
                        out_ap = nxt[:] if j == C - 1 else a1[:]
                        vec.tensor_tensor(out=out_ap, in0=a1[:], in1=t1[:],
                                          op=ALU.max)
                cur, nxt = nxt, cur
            m1 = cur

            msa = pool.tile([128, C], F32)
            msb = pool.tile([128, C], F32)
            t2 = pool.tile([128, C], F32)
            a2 = pool.tile([128, C], F32)
            vec.tensor_copy(out=msa[:], in_=v(einit1[:], 0,
                                              [P(einit1), [KL, C], [1, 1]]))
            cur2, nxt2 = msa, msb
            for g in range(NG - 1, -1, -1):
                for j in range(C):
                    dst = a2[:] if j == 0 else t2[:]
                    vec.scalar_tensor_tensor(
                        out=dst, in0=cur2[:], scalar=float(j + 1),
                        in1=v(m1[:], j * NG + g, [P(m1), [0, C], [0, 1]]),
                        op0=ALU.is_equal, op1=ALU.mult)
                    if j > 0:
                        out_ap = nxt2[:] if j == C - 1 else a2[:]
                        vec.tensor_tensor(out=out_ap, in0=a2[:], in1=t2[:],
                                          op=ALU.max)
                cur2, nxt2 = nxt2, cur2
            nc.sync.dma_start(out=smap_d[:], in_=cur2[:])
            tc.strict_bb_all_engine_barrier()

            smap_t = pool.tile([SEQ, NSTRIP * C], F32)
            nc.sync.dma_start(out=smap_t[:],
                              in_=v(smap_d, 0, [[C, SEQ], [C * SEQ, NSTRIP],
                                                [1, C]]))
            state = pool.tile([SEQ, 1], F32)
            sacc = pool.tile([SEQ, 1], F32)
            stmp = pool.tile([SEQ, 1], F32)
            estrip = pool.tile([SEQ, NSTRIP], F32)
            vec.memset(state[:], 1.0)
            for sg in range(NSTRIP - 1, -1, -1):
                vec.tensor_copy(out=estrip[:, sg:sg + 1], in_=state[:])
                for j in range(C):
                    dst = sacc if j == 0 else stmp
                    vec.scalar_tensor_tensor(
                        out=dst[:], in0=state[:], scalar=float(j + 1),
                        in1=smap_t[:, sg * C + j:sg * C + j + 1],
                        op0=ALU.is_equal, op1=ALU.mult)
                    if j > 0:
                        out_ap = state[:] if j == C - 1 else sacc[:]
                        vec.tensor_tensor(out=out_ap, in0=sacc[:], in1=stmp[:],
                                          op=ALU.max)
            nc.sync.dma_start(out=estrip_d, in_=estrip[:])
            tc.strict_bb_all_engine_barrier()
            eseed = pool.tile([128, 1], F32)
            nc.sync.dma_start(out=eseed[:],
                              in_=v(estrip_d, 0, [[1, NSTRIP], [NSTRIP, SEQ],
                                                  [1, 1]]))

            eg = pool.tile([128, NG], F32)
            st2 = pool.tile([128, 1], F32)
            d2a = pool.tile([128, 1], F32)
            d2t = pool.tile([128, 1], F32)
            vec.tensor_copy(out=st2[:], in_=eseed[:])
            for g in range(NG - 1, -1, -1):
                vec.tensor_copy(out=eg[:, g:g + 1], in_=st2[:])
                for j in range(C):
                    dst = d2a if j == 0 else d2t
                    vec.scalar_tensor_tensor(
                        out=dst[:], in0=st2[:], scalar=float(j + 1),
                        in1=v(m1[:], j * NG + g, [P(m1), [0, 1]]),
                        op0=ALU.is_equal, op1=ALU.mult)
                    if j > 0:
                        out_ap = st2[:] if j == C - 1 else d2a[:]
                        vec.tensor_tensor(out=out_ap, in0=d2a[:], in1=d2t[:],
                                          op=ALU.max)

            ek = pool.tile([128, KL], F32)
            st3 = pool.tile([128, NG], F32)
            d1a = pool.tile([128, NG], F32)
            d1t = pool.tile([128, NG], F32)
            vec.tensor_copy(out=st3[:], in_=eg[:])
            for kk in range(G1 - 1, -1, -1):
                vec.tensor_copy(out=v(ek[:], kk, [P(ek), [G1, NG]]), in_=st3[:])
                for j in range(C):
                    dst = d1a if j == 0 else d1t
                    vec.scalar_tensor_tensor(
                        out=dst[:], in0=st3[:], scalar=float(j + 1),
                        in1=v(cand1[:], j * KL * S + kk * S,
                              [P(cand1), [G1 * S, NG]]),
                        op0=ALU.is_equal, op1=ALU.mult)
                    if j > 0:
                        out_ap = st3[:] if j == C - 1 else d1a[:]
                        vec.tensor_tensor(out=out_ap, in0=d1a[:], in1=d1t[:],
                                          op=ALU.max)

            acc2 = pool.tile([128, STRIP], F32)
            tsel = pool.tile([128, STRIP], F32)
            for e in range(C):
                dst = acc2 if e == 0 else tsel
                vec.scalar_tensor_tensor(
                    out=dst[:],
                    in0=v(ek[:], 0, [P(ek), [1, KL], [0, S]]),
                    scalar=float(e + 1),
                    in1=v(cand1[:], e * KL * S, [P(cand1), [S, KL], [1, S]]),
                    op0=ALU.is_equal, op1=ALU.mult)
                if e > 0:
                    vec.tensor_tensor(out=acc2[:], in0=acc2[:], in1=tsel[:],
                                      op=ALU.max)
            mask = pool.tile([128, STRIP], F32)
            vec.tensor_scalar(out=mask[:], in0=tplane[:], scalar1=len_sb[:],
                              scalar2=None, op0=ALU.is_lt)
            vec.tensor_tensor(out=acc2[:], in0=acc2[:], in1=mask[:],
                              op=ALU.mult)
            vec.tensor_scalar(out=acc2[:], in0=acc2[:], scalar1=-1.0,
                              scalar2=None, op0=ALU.add)
            path_i = pool.tile([128, STRIP], I32)
            vec.tensor_copy(out=path_i[:], in_=acc2[:])
            nc.sync.dma_start(out=path_d, in_=path_i[:])
        bp_pool.release()


def _build(L):
    import concourse.bacc as bacc
    import concourse.mybir as mybir
    from concourse import tile

    _register_crf_op()
    sh = _shapes(L)
    nc = bacc.Bacc("TRN2", target_bir_lowering=False, debug=False,
                   num_devices=NCORES)
    F32 = mybir.dt.float32
    ins_aps = {
        "obs_sc": nc.dram_tensor("obs_sc", [SEQ, (L + 2 * TBLK) * C], F32,
                                 kind="ExternalInput").ap(),
        "ot_init": nc.dram_tensor("ot_init", [SEQ, TBLK * PAGE], F32,
                                  kind="ExternalInput").ap(),
        "trep": nc.dram_tensor("trep", [128, C * C], F32,
                               kind="ExternalInput").ap(),
        "wdesc": nc.dram_tensor("wdesc", [128, C], F32,
                                kind="ExternalInput").ap(),
        "tplane": nc.dram_tensor("tplane", [128, sh["STRIP"]], F32,
                                 kind="ExternalInput").ap(),
        "len_col": nc.dram_tensor("len_col", [128, 1], F32,
                                  kind="ExternalInput").ap(),
        "lenm1": nc.dram_tensor("lenm1", [128, 1], F32,
                                kind="ExternalInput").ap(),
        "einit1": nc.dram_tensor("einit1", [128, C * sh["KL"]], F32,
                                 kind="ExternalInput").ap(),
    }
    outs_aps = {"path": nc.dram_tensor("path", [128, sh["STRIP"]],
                                       mybir.dt.int32,
                                       kind="ExternalOutput").ap()}
    with tile.TileContext(nc) as tc:
        _emit(tc, ins_aps, outs_aps, L)
    nc.compile()
    return nc


def kernel(observes, transitions, lengths):
    from concourse.bass_utils import run_bass_kernel_spmd

    observes = np.asarray(observes, np.float32)
    transitions = np.asarray(transitions, np.float32)
    lengths_np = np.asarray(lengths)
    L = observes.shape[2]

    if L not in _CACHE:
        _CACHE[L] = _build(L)
    nc = _CACHE[L]

    in_maps = [
        _host_prep(observes[SEQ * c:SEQ * (c + 1)], transitions,
                   lengths_np[SEQ * c:SEQ * (c + 1)], L)
        for c in range(NCORES)
    ]
    res = run_bass_kernel_spmd(nc, in_maps, core_ids=list(range(NCORES)))
    out = np.concatenate(
        [_host_post(res.results[c]["path"], L) for c in range(NCORES)], 0)
    return out.astype(np.int32)


# revision 4
# speedup vs baseline: 1.2836x; 1.0001x over previous
"""Batched CRF Viterbi decode (N=64, C=8, L=32768) on 8 TRN2 NeuronCores.

Self-contained kernel: takes FULL unsharded inputs, shards the batch dim
across 8 cores (data-parallel), runs a Bass/Tile kernel per core, and
gathers the full [64, 32768] int32 path.

Phase 1 uses a hand-built custom DVE op (CRF_STEP_ANT): ONE instruction
per Viterbi timestep (vs 3 stock ops), computing
  fv_t[i] = max_j fl(fl(fv_{t-1}[j] + T[i,j]) + obs_t[i])
which by monotonicity of fp rounding equals the reference's
fl(max_j fl(fv+T) + obs) bit-exactly.  Phases 2/3 (backpointer
extraction + backward traversal) are bulk-parallel stock ops.
"""
import sys
import numpy as np

if '/opt/trn_rl_repo' not in sys.path:
    sys.path.insert(0, '/opt/trn_rl_repo')

N_FULL, C, L = 64, 8, 32768
SEQ = 8          # sequences per core
NSTRIP = 16      # time strips per core for phases 2/3 (partition dim 128)
S = 16           # phase-3 chunk length
NCORES = 8
TBLK = 256       # phase-1 steps per block
PAGE = 9 * C     # 72 elems per step in the OT stream

_CACHE = {}

# ---------------------------------------------------------------- custom op
OP_NAME = "CRF_STEP_ANT"


def _register_crf_op():
    from concourse.dve_uop import (
        ENABLE, AluInp, AluOp, InpSel, OutPath, OutSel, Trigger, UopConfig,
        DveOpSpec,
    )
    from concourse.dve_spec import Spec, Src0, Src1
    from concourse import dve_ops

    if OP_NAME in dve_ops._SUB_OPCODE_FOR_NAME:
        return dve_ops._CRF_STEP_OP

    def _dp_tail(u):
        for k in range(2, 8):
            u.datapath_config[k].pass_through_alu()
        return u

    def _mk_steady():
        u = UopConfig()
        u.enable_input(InpSel.SRC_0, 0)
        u.enable_input(InpSel.SRC_1, 1)
        u.datapath_config[0].enable_alu(AluOp.ADD, AluInp.PREV_ALU_OUT,
                                        AluInp.PREV_DELAY_0)
        u.datapath_config[1].enable_alu(AluOp.MAX, AluInp.CURR_ALU_OUT,
                                        AluInp.PREV_ALU_OUT)
        u.require_inp0 = ENABLE
        u.require_inp1 = ENABLE
        u.repeat_count = C - 1
        u.trigger = (Trigger.SRC_TENSOR_DONE, Trigger.COUNT, Trigger.NONE)
        u.next_uop = (0, 2, 0)
        return _dp_tail(u)

    def _mk_obs():
        u = UopConfig()
        u.enable_input(InpSel.SRC_0, 0)
        u.enable_input(InpSel.SRC_1, 1)
        u.datapath_config[0].enable_alu(AluOp.BYPASS, AluInp.PREV_ALU_OUT)
        u.datapath_config[1].enable_alu(AluOp.ADD, AluInp.CURR_ALU_OUT,
                                        AluInp.PREV_ALU_OUT)
        u.enable_output(OutSel.ALU_OUT, OutPath.WR0_LO)
        u.require_inp0 = ENABLE
        u.require_inp1 = ENABLE
        u.repeat_count = 1
        u.trigger = (Trigger.SRC_TENSOR_DONE, Trigger.COUNT, Trigger.NONE)
        u.next_uop = (0, 3, 0)
        return _dp_tail(u)

    def _mk_reset():
        u = UopConfig()
        u.enable_input(InpSel.SRC_0, 0)
        u.enable_input(InpSel.SRC_1, 1)
        u.enable_input(InpSel.MAX_NEG, 2)
        u.datapath_config[0].enable_alu(AluOp.ADD, AluInp.PREV_ALU_OUT,
                                        AluInp.PREV_DELAY_0)
        u.datapath_config[0].pass_through_delay(1)
        u.datapath_config[1].enable_alu(AluOp.MAX, AluInp.PREV_DELAY_1,
                                        AluInp.PREV_ALU_OUT)
        u.require_inp0 = ENABLE
        u.require_inp1 = ENABLE
        u.repeat_count = 1
        u.trigger = (Trigger.SRC_TENSOR_DONE, Trigger.COUNT, Trigger.NONE)
        u.next_uop = (0, 1, 0)
        return _dp_tail(u)

    class CrfStepOp:
        name = OP_NAME
        subdim = False
        spec = Spec(body=Src0 + Src1, reference=lambda in0, in1: in0 + in1)

        def __init__(self):
            self._cache = {}

        def compile(self, ver):
            if ver not in self._cache:
                s = DveOpSpec(
                    name=self.name,
                    uops=[_mk_reset(), _mk_steady(), _mk_obs(), _mk_reset()],
                    opcode=dve_ops.get_dve_sub_opcode(self.name),
                    rd1_en=True,
                )
                s.validate(ver)
                self._cache[ver] = s
            return self._cache[ver]

    op = CrfStepOp()
    row = max(dve_ops._SUB_OPCODE_FOR_NAME.values()) + 1
    assert row < 0x20
    dve_ops._SUB_OPCODE_FOR_NAME[OP_NAME] = row
    dve_ops.OPS.append(op)
    dve_ops.CUSTOM_DVE_SPECS[OP_NAME] = op.spec
    dve_ops._CRF_STEP_OP = op
    return op


# ---------------------------------------------------------------- host side
def _shapes(L):
    STRIP = L // NSTRIP
    TB = min(128, STRIP)
    return dict(STRIP=STRIP, TB=TB, ROUNDS=STRIP // TB, KL=STRIP // S)


def _host_prep(observes_core, transitions, lengths_core, L):
    sh = _shapes(L)
    STRIP, KL = sh["STRIP"], sh["KL"]
    obs_t = np.ascontiguousarray(
        np.transpose(np.asarray(observes_core, np.float32), (0, 2, 1)))
    # scatter source: [s, t, i] contiguous + 2 blocks of zero pad
    obs_sc = np.zeros((SEQ, (L + 2 * TBLK) * C), np.float32)
    obs_sc[:, :L * C] = obs_t.reshape(SEQ, L * C)
    T = np.asarray(transitions, np.float32)
    # OT image for one block: page i of step t = [T[i,0..7], 0]
    ot1 = np.zeros((SEQ, TBLK * PAGE), np.float32)
    trow = np.zeros(PAGE, np.float32)
    for i in range(C):
        trow[i * 9:i * 9 + 8] = T[i, :]
    ot1[:, :] = np.tile(trow, TBLK)[None, :]
    lens = np.asarray(lengths_core).astype(np.float32)
    p = np.arange(128)
    return {
        "obs_sc": obs_sc,
        "ot_init": ot1,
        "trep": np.tile(T.reshape(1, C * C), (128, 1)).astype(np.float32),
        "wdesc": np.tile((C - np.arange(C, dtype=np.float32)).reshape(1, C),
                         (128, 1)),
        "tplane": ((p[:, None] // SEQ) * STRIP
                   + np.arange(STRIP)[None, :]).astype(np.float32),
        "len_col": lens[p % SEQ][:, None].astype(np.float32),
        "lenm1": (lens[p % SEQ][:, None] - 1.0).astype(np.float32),
        "einit1": np.tile((np.arange(C, dtype=np.float32)[:, None] + 1.0),
                          (1, KL)).reshape(1, C * KL).repeat(128, 0)
                    .astype(np.float32),
    }


def _host_post(path_dev, L):
    STRIP = L // NSTRIP
    return path_dev.reshape(NSTRIP, SEQ, STRIP).transpose(1, 0, 2).reshape(SEQ, L)


# ---------------------------------------------------------------- device
def _emit(tc, ins, outs, L):
    import concourse.bass as bass
    import concourse.mybir as mybir
    import bass_rust
    from concourse import dve_ops

    F32 = mybir.dt.float32
    I32 = mybir.dt.int32
    ALU = mybir.AluOpType
    AX = mybir.AxisListType

    crf_op = dve_ops._CRF_STEP_OP

    def v(ap, off, dims):
        return bass_rust.AP(tensor=ap.tensor, offset=ap.offset + off, ap=dims)

    nc = tc.nc
    sh = _shapes(L)
    STRIP, TB, ROUNDS, KL = sh["STRIP"], sh["TB"], sh["ROUNDS"], sh["KL"]
    G1 = min(8, KL)
    NG = KL // G1
    FLATN = (L + 1) * C
    NBLK = L // TBLK

    obs_d = ins["obs_sc"]
    ot_init_d = ins["ot_init"]
    trep_d = ins["trep"]
    wdesc_d = ins["wdesc"]
    tplane_d = ins["tplane"]
    len_d = ins["len_col"]
    lenm1_d = ins["lenm1"]
    einit1_d = ins["einit1"]
    path_d = outs["path"]

    fv_d = nc.dram_tensor("fv_scratch", [SEQ, FLATN], F32).ap()
    smap_d = nc.dram_tensor("smap_scratch", [128, C], F32).ap()
    estrip_d = nc.dram_tensor("estrip_scratch", [SEQ, NSTRIP], F32).ap()

    vec = nc.vector

    with tc.tile_pool(name="const", bufs=1) as cpool:
        trep = cpool.tile([128, C * C], F32)
        wdesc = cpool.tile([128, C], F32)
        tplane = cpool.tile([128, STRIP], F32)
        len_sb = cpool.tile([128, 1], F32)
        lenm1_sb = cpool.tile([128, 1], F32)
        nc.sync.dma_start(out=trep[:], in_=trep_d)
        nc.sync.dma_start(out=wdesc[:], in_=wdesc_d)
        nc.sync.dma_start(out=tplane[:], in_=tplane_d)
        nc.sync.dma_start(out=len_sb[:], in_=len_d)
        nc.sync.dma_start(out=lenm1_sb[:], in_=lenm1_d)

        # ============ phase 1: fused custom-op chain ============
        with tc.tile_pool(name="ph1", bufs=1) as pool:
            otA = pool.tile([SEQ, TBLK * PAGE], F32)
            otB = pool.tile([SEQ, TBLK * PAGE], F32)
            fvA = pool.tile([SEQ, TBLK * C + C], F32)
            fvB = pool.tile([SEQ, TBLK * C + C], F32)
            vec.memset(fvA[:], 0.0)
            vec.memset(fvB[:], 0.0)
            nc.sync.dma_start(out=otA[:], in_=ot_init_d)
            nc.sync.dma_start(out=otB[:], in_=ot_init_d)
            # fv_d col 0 = fv_init = 0
            nc.sync.dma_start(out=v(fv_d, 0, [[FLATN, SEQ], [1, C]]),
                              in_=fvA[:, 0:C])

            P = lambda t: t[:].ap[0]
            SC_SB = [[PAGE, TBLK], [9, C]]     # obs slots in an OT tile

            def steps(ot_t, fv_t, fv_prev_t):
                for t in range(TBLK):
                    src = (v(fv_prev_t[:], (TBLK - 1) * C,
                             [P(fv_prev_t), [0, C], [1, 9]]) if t == 0 else
                           v(fv_t[:], (t - 1) * C, [P(fv_t), [0, C], [1, 9]]))
                    nc.vector._custom_dve(
                        crf_op,
                        out=fv_t[:, t * C:(t + 1) * C],
                        in0=v(ot_t[:], t * PAGE, [P(ot_t), [9, C], [1, 9]]),
                        in1=src)

            # prologue: scatter obs for block 0 into otA
            nc.sync.dma_start(out=v(otA[:], C, [P(otA)] + SC_SB),
                              in_=obs_d[:, 0:TBLK * C])
            BB = TBLK * C  # elems per block in obs_sc AND in fv_d cols
            OBS_ROW = (L + 2 * TBLK) * C
            obs_p1 = v(obs_d, BB, [[OBS_ROW, SEQ], [1, OBS_ROW - BB]])
            obs_p2 = v(obs_d, 2 * BB, [[OBS_ROW, SEQ], [1, OBS_ROW - 2 * BB]])
            fv_lo = v(fv_d, C, [[FLATN, SEQ], [1, FLATN - C]])
            fv_hi = v(fv_d, C + BB, [[FLATN, SEQ], [1, FLATN - C - BB]])
            with tc.For_i(0, (NBLK - 2) * BB, 2 * BB,
                          hint_engines=(mybir.EngineType.DVE,),
                          staggered_reset=True) as iofs:
                nc.sync.dma_start(out=v(otB[:], C, [P(otB)] + SC_SB),
                                  in_=obs_p1[:, bass.ds(iofs, BB)])
                steps(otA, fvA, fvB)
                nc.sync.dma_start(out=fv_lo[:, bass.ds(iofs, BB)],
                                  in_=fvA[:, 0:BB])
                nc.sync.dma_start(out=v(otA[:], C, [P(otA)] + SC_SB),
                                  in_=obs_p2[:, bass.ds(iofs, BB)])
                steps(otB, fvB, fvA)
                nc.sync.dma_start(out=fv_hi[:, bass.ds(iofs, BB)],
                                  in_=fvB[:, 0:BB])
            # epilogue: last two blocks with static offsets
            lo = (NBLK - 2) * BB
            nc.sync.dma_start(out=v(otB[:], C, [P(otB)] + SC_SB),
                              in_=obs_d[:, lo + BB:lo + 2 * BB])
            steps(otA, fvA, fvB)
            nc.sync.dma_start(out=v(fv_d, C + lo, [[FLATN, SEQ], [1, BB]]),
                              in_=fvA[:, 0:BB])
            steps(otB, fvB, fvA)
            nc.sync.dma_start(out=v(fv_d, C + lo + BB,
                                    [[FLATN, SEQ], [1, BB]]),
                              in_=fvB[:, 0:BB])

        # ============ phase 2: backpointer extraction ============
        # bp stays in SBUF end-to-end: the 8 MB DRAM round-trip costs ~1 ms
        # of sync-queue descriptor generation alone.
        bp_pool = tc.alloc_tile_pool(name="bp", bufs=1)
        bp_sb = bp_pool.tile([128, STRIP * C], F32)
        fvpool = tc.alloc_tile_pool(name="ph2fv", bufs=3)
        with tc.tile_pool(name="ph2", bufs=1) as pool:
            for r in range(ROUNDS):
                off = r * TB * C
                fv_blk = fvpool.tile([128, (TB + 1) * C], F32, tag="fv",
                                     name=f"fvblk{r}")
                src_dims = [[STRIP * C, NSTRIP], [FLATN, SEQ],
                            [1, (TB + 1) * C]]
                nc.sync.dma_start(out=fv_blk[:], in_=v(fv_d, off, src_dims))

                P = lambda t: t[:].ap[0]
                sc2 = pool.tile([128, C * TB * C], F32, tag="sc")
                eq2 = pool.tile([128, C * TB * C], F32, tag="eq")
                vitr = pool.tile([128, C * TB], F32, tag="vitr")
                vec.tensor_tensor(
                    out=sc2[:],
                    in0=v(fv_blk[:], 0, [P(fv_blk), [0, C], [C, TB], [1, C]]),
                    in1=v(trep[:], 0, [P(trep), [C, C], [0, TB], [1, C]]),
                    op=ALU.add)
                vec.tensor_reduce(
                    out=vitr[:],
                    in_=v(sc2[:], 0, [P(sc2), [TB * C, C], [C, TB], [1, C]]),
                    axis=AX.X, op=ALU.max)
                vec.tensor_tensor(
                    out=eq2[:],
                    in0=v(sc2[:], 0, [P(sc2), [TB * C, C], [C, TB], [1, C]]),
                    in1=v(vitr[:], 0, [P(vitr), [TB, C], [1, TB], [0, C]]),
                    op=ALU.is_equal)
                vec.tensor_tensor(
                    out=eq2[:],
                    in0=v(eq2[:], 0, [P(eq2), [TB * C, C], [C, TB], [1, C]]),
                    in1=v(wdesc[:], 0, [P(wdesc), [0, C], [0, TB], [1, C]]),
                    op=ALU.mult)
                bpw = pool.tile([128, C * TB], F32, tag="bpw")
                vec.tensor_reduce(
                    out=bpw[:],
                    in_=v(eq2[:], 0, [P(eq2), [TB * C, C], [C, TB], [1, C]]),
                    axis=AX.X, op=ALU.max)
                bp1 = pool.tile([128, C * TB], F32, tag="bp1")
                vec.tensor_scalar(out=bp1[:], in0=bpw[:], scalar1=-1.0,
                                  scalar2=9.0, op0=ALU.mult, op1=ALU.add)

                fm = pool.tile([128, TB], F32, tag="fm")
                vec.tensor_reduce(
                    out=fm[:],
                    in_=v(fv_blk[:], C, [P(fv_blk), [C, TB], [1, C]]),
                    axis=AX.X, op=ALU.max)
                eqn = pool.tile([128, TB * C], F32, tag="eqn")
                vec.tensor_tensor(
                    out=eqn[:],
                    in0=v(fv_blk[:], C, [P(fv_blk), [C, TB], [1, C]]),
                    in1=v(fm[:], 0, [P(fm), [1, TB], [0, C]]),
                    op=ALU.is_equal)
                vec.tensor_tensor(
                    out=eqn[:],
                    in0=v(eqn[:], 0, [P(eqn), [C, TB], [1, C]]),
                    in1=v(wdesc[:], 0, [P(wdesc), [0, TB], [1, C]]),
                    op=ALU.mult)
                mn = pool.tile([128, TB], F32, tag="mn")
                vec.tensor_reduce(
                    out=mn[:],
                    in_=v(eqn[:], 0, [P(eqn), [C, TB], [1, C]]),
                    axis=AX.X, op=ALU.max)
                en1 = pool.tile([128, TB], F32, tag="en1")
                vec.tensor_scalar(out=en1[:], in0=mn[:], scalar1=-1.0,
                                  scalar2=9.0, op0=ALU.mult, op1=ALU.add)
                endsel = pool.tile([128, TB], F32, tag="endsel")
                tmp = pool.tile([128, TB], F32, tag="tmpsel")
                for j in range(C):
                    dst = endsel if j == 0 else tmp
                    vec.scalar_tensor_tensor(
                        out=dst[:], in0=en1[:], scalar=float(j + 1),
                        in1=bp1[:, j * TB:(j + 1) * TB],
                        op0=ALU.is_equal, op1=ALU.mult)
                    if j > 0:
                        vec.tensor_tensor(out=endsel[:], in0=endsel[:],
                                          in1=tmp[:], op=ALU.max)
                atm = pool.tile([128, TB], F32, tag="atm")
                vec.tensor_scalar(out=atm[:], in0=tplane[:, r * TB:(r + 1) * TB],
                                  scalar1=lenm1_sb[:], scalar2=None,
                                  op0=ALU.is_equal)
                dsel = pool.tile([128, TB * C], F32, tag="dsel")
                bp1_tn = v(bp1[:], 0, [P(bp1), [1, TB], [TB, C]])
                vec.tensor_tensor(
                    out=dsel[:],
                    in0=v(endsel[:], 0, [P(endsel), [1, TB], [0, C]]),
                    in1=bp1_tn, op=ALU.subtract)
                vec.tensor_tensor(
                    out=dsel[:],
                    in0=v(dsel[:], 0, [P(dsel), [C, TB], [1, C]]),
                    in1=v(atm[:], 0, [P(atm), [1, TB], [0, C]]),
                    op=ALU.mult)
                vec.tensor_tensor(out=bp_sb[:, off:off + TB * C],
                                  in0=bp1_tn, in1=dsel[:], op=ALU.add)

        fvpool.release()
        tc.strict_bb_all_engine_barrier()

        # ============ phase 3: chunked backward ============
        with tc.tile_pool(name="ph3", bufs=1) as pool:
            P = lambda t: t[:].ap[0]
            bp_strip = bp_sb
            einit1 = pool.tile([128, C * KL], F32)
            nc.sync.dma_start(out=einit1[:], in_=einit1_d)
            cand1 = pool.tile([128, C * KL * S], F32)
            acc = pool.tile([128, C * KL], F32)
            tmp = pool.tile([128, C * KL], F32)

            def cand_col(tl):
                return v(cand1[:], tl, [P(cand1), [KL * S, C], [S, KL]])

            for tl in range(S - 1, -1, -1):
                if tl == S - 1:
                    prev = v(einit1[:], 0, [P(einit1), [KL, C], [1, KL]])
                else:
                    prev = cand_col(tl + 1)
                for j in range(C):
                    dst = acc[:] if j == 0 else tmp[:]
                    vec.scalar_tensor_tensor(
                        out=dst, in0=prev, scalar=float(j + 1),
                        in1=v(bp_strip[:], tl * C + j,
                              [P(bp_strip), [0, C], [S * C, KL]]),
                        op0=ALU.is_equal, op1=ALU.mult)
                    if j > 0:
                        out_ap = cand_col(tl) if j == C - 1 else acc[:]
                        vec.tensor_tensor(out=out_ap, in0=acc[:], in1=tmp[:],
                                          op=ALU.max)

            m1a = pool.tile([128, C * NG], F32)
            m1b = pool.tile([128, C * NG], F32)
            t1 = pool.tile([128, C * NG], F32)
            a1 = pool.tile([128, C * NG], F32)
            vec.tensor_copy(out=m1a[:],
                            in_=v(einit1[:], 0, [P(einit1), [KL, C], [G1, NG]]))
            cur, nxt = m1a, m1b
            for kk in range(G1 - 1, -1, -1):
                for j in range(C):
                    dst = a1[:] if j == 0 else t1[:]
                    vec.scalar_tensor_tensor(
                        out=dst, in0=cur[:], scalar=float(j + 1),
                        in1=v(cand1[:], j * KL * S + kk * S,
                              [P(cand1), [0, C], [G1 * S, NG]]),
                        op0=ALU.is_equal, op1=ALU.mult)
                    if j > 0:
                        out_ap = nxt[:] if j == C - 1 else a1[:]
                        vec.tensor_tensor(out=out_ap, in0=a1[:], in1=t1[:],
                                          op=ALU.max)
                cur, nxt = nxt, cur
            m1 = cur

            msa = pool.tile([128, C], F32)
            msb = pool.tile([128, C], F32)
            t2 = pool.tile([128, C], F32)
            a2 = pool.tile([128, C], F32)
            vec.tensor_copy(out=msa[:], in_=v(einit1[:], 0,
                                              [P(einit1), [KL, C], [1, 1]]))
            cur2, nxt2 = msa, msb
            for g in range(NG - 1, -1, -1):
                for j in range(C):
                    dst = a2[:] if j == 0 else t2[:]
                    vec.scalar_tensor_tensor(
                        out=dst, in0=cur2[:], scalar=float(j + 1),
                        in1=v(m1[:], j * NG + g, [P(m1), [0, C], [0, 1]]),
                        op0=ALU.is_equal, op1=ALU.mult)
                    if j > 0:
                        out_ap = nxt2[:] if j == C - 1 else a2[:]
                        vec.tensor_tensor(out=out_ap, in0=a2[:], in1=t2[:],
                                          op=ALU.max)
                cur2, nxt2 = nxt2, cur2
            nc.sync.dma_start(out=smap_d[:], in_=cur2[:])
            tc.strict_bb_all_engine_barrier()

            smap_t = pool.tile([SEQ, NSTRIP * C], F32)
            nc.sync.dma_start(out=smap_t[:],
                              in_=v(smap_d, 0, [[C, SEQ], [C * SEQ, NSTRIP],
                                                [1, C]]))
            state = pool.tile([SEQ, 1], F32)
            sacc = pool.tile([SEQ, 1], F32)
            stmp = pool.tile([SEQ, 1], F32)
            estrip = pool.tile([SEQ, NSTRIP], F32)
            vec.memset(state[:], 1.0)
            for sg in range(NSTRIP - 1, -1, -1):
                vec.tensor_copy(out=estrip[:, sg:sg + 1], in_=state[:])
                for j in range(C):
                    dst = sacc if j == 0 else stmp
                    vec.scalar_tensor_tensor(
                        out=dst[:], in0=state[:], scalar=float(j + 1),
                        in1=smap_t[:, sg * C + j:sg * C + j + 1],
                        op0=ALU.is_equal, op1=ALU.mult)
                    if j > 0:
                        out_ap = state[:] if j == C - 1 else sacc[:]
                        vec.tensor_tensor(out=out_ap, in0=sacc[:], in1=stmp[:],
                                          op=ALU.max)
            nc.sync.dma_start(out=estrip_d, in_=estrip[:])
            tc.strict_bb_all_engine_barrier()
            eseed = pool.tile([128, 1], F32)
            nc.sync.dma_start(out=eseed[:],
                              in_=v(estrip_d, 0, [[1, NSTRIP], [NSTRIP, SEQ],
                                                  [1, 1]]))

            eg = pool.tile([128, NG], F32)
            st2 = pool.tile([128, 1], F32)
            d2a = pool.tile([128, 1], F32)
            d2t = pool.tile([128, 1], F32)
            vec.tensor_copy(out=st2[:], in_=eseed[:])
            for g in range(NG - 1, -1, -1):
                vec.tensor_copy(out=eg[:, g:g + 1], in_=st2[:])
                for j in range(C):
                    dst = d2a if j == 0 else d2t
                    vec.scalar_tensor_tensor(
                        out=dst[:], in0=st2[:], scalar=float(j + 1),
                        in1=v(m1[:], j * NG + g, [P(m1), [0, 1]]),
                        op0=ALU.is_equal, op1=ALU.mult)
                    if j > 0:
                        out_ap = st2[:] if j == C - 1 else d2a[:]
                        vec.tensor_tensor(out=out_ap, in0=d2a[:], in1=d2t[:],
                                          op=ALU.max)

            ek = pool.tile([128, KL], F32)
            st3 = pool.tile([128, NG], F32)
            d1a = pool.tile([128, NG], F32)
            d1t = pool.tile([128, NG], F32)
            vec.tensor_copy(out=st3[:], in_=eg[:])
            for kk in range(G1 - 1, -1, -1):
                vec.tensor_copy(out=v(ek[:], kk, [P(ek), [G1, NG]]), in_=st3[:])
                for j in range(C):
                    dst = d1a if j == 0 else d1t
                    vec.scalar_tensor_tensor(
                        out=dst[:], in0=st3[:], scalar=float(j + 1),
                        in1=v(cand1[:], j * KL * S + kk * S,
                              [P(cand1), [G1 * S, NG]]),
                        op0=ALU.is_equal, op1=ALU.mult)
                    if j > 0:
                        out_ap = st3[:] if j == C - 1 else d1a[:]
                        vec.tensor_tensor(out=out_ap, in0=d1a[:], in1=d1t[:],
                                          op=ALU.max)

            acc2 = pool.tile([128, STRIP], F32)
            tsel = pool.tile([128, STRIP], F32)
            for e in range(C):
                dst = acc2 if e == 0 else tsel
                vec.scalar_tensor_tensor(
                    out=dst[:],
                    in0=v(ek[:], 0, [P(ek), [1, KL], [0, S]]),
                    scalar=float(e + 1),
                    in1=v(cand1[:], e * KL * S, [P(cand1), [S, KL], [1, S]]),
                    op0=ALU.is_equal, op1=ALU.mult)
                if e > 0:
                    vec.tensor_tensor(out=acc2[:], in0=acc2[:], in1=tsel[:],
                                      op=ALU.max)
            mask = pool.tile([128, STRIP], F32)
            vec.tensor_scalar(out=mask[:], in0=tplane[:], scalar1=len_sb[:],
                              scalar2=None, op0=ALU.is_lt)
            vec.tensor_tensor(out=acc2[:], in0=acc2[:], in1=mask[:],
                              op=ALU.mult)
            vec.tensor_scalar(out=acc2[:], in0=acc2[:], scalar1=-1.0,
                              scalar2=None, op0=ALU.add)
            path_i = pool.tile([128, STRIP], I32)
            vec.tensor_copy(out=path_i[:], in_=acc2[:])
            nc.sync.dma_start(out=path_d, in_=path_i[:])
        bp_pool.release()


def _build(L):
    import concourse.bacc as bacc
    import concourse.mybir as mybir
    from concourse import tile

    _register_crf_op()
    sh = _shapes(L)
    nc = bacc.Bacc("TRN2", target_bir_lowering=False, debug=False,
                   num_devices=NCORES)
    F32 = mybir.dt.float32
    ins_aps = {
        "obs_sc": nc.dram_tensor("obs_sc", [SEQ, (L + 2 * TBLK) * C], F32,
                                 kind="ExternalInput").ap(),
        "ot_init": nc.dram_tensor("ot_init", [SEQ, TBLK * PAGE], F32,
                                  kind="ExternalInput").ap(),
        "trep": nc.dram_tensor("trep", [128, C * C], F32,
                               kind="ExternalInput").ap(),
        "wdesc": nc.dram_tensor("wdesc", [128, C], F32,
                                kind="ExternalInput").ap(),
        "tplane": nc.dram_tensor("tplane", [128, sh["STRIP"]], F32,
                                 kind="ExternalInput").ap(),
        "len_col": nc.dram_tensor("len_col", [128, 1], F32,
                                  kind="ExternalInput").ap(),
        "lenm1": nc.dram_tensor("lenm1", [128, 1], F32,
                                kind="ExternalInput").ap(),
        "einit1": nc.dram_tensor("einit1", [128, C * sh["KL"]], F32,
                                 kind="ExternalInput").ap(),
    }
    outs_aps = {"path": nc.dram_tensor("path", [128, sh["STRIP"]],
                                       mybir.dt.int32,
                                       kind="ExternalOutput").ap()}
    with tile.TileContext(nc) as tc:
        _emit(tc, ins_aps, outs_aps, L)
    nc.compile()
    return nc


def kernel(observes, transitions, lengths):
    from concourse.bass_utils import run_bass_kernel_spmd

    observes = np.asarray(observes, np.float32)
    transitions = np.asarray(transitions, np.float32)
    lengths_np = np.asarray(lengths)
    L = observes.shape[2]

    if L not in _CACHE:
        _CACHE[L] = _build(L)
    nc = _CACHE[L]

    in_maps = [
        _host_prep(observes[SEQ * c:SEQ * (c + 1)], transitions,
                   lengths_np[SEQ * c:SEQ * (c + 1)], L)
        for c in range(NCORES)
    ]
    res = run_bass_kernel_spmd(nc, in_maps, core_ids=list(range(NCORES)))
    out = np.concatenate(
        [_host_post(res.results[c]["path"], L) for c in range(NCORES)], 0)
    return out.astype(np.int32)


# revision 5
# speedup vs baseline: 1.2841x; 1.0004x over previous
"""Batched CRF Viterbi decode (N=64, C=8, L=32768) on 8 TRN2 NeuronCores.

Self-contained kernel: takes FULL unsharded inputs, shards the batch dim
across 8 cores (data-parallel), runs a Bass/Tile kernel per core, and
gathers the full [64, 32768] int32 path.

Phase 1 uses a hand-built custom DVE op (CRF_STEP_ANT): ONE instruction
per Viterbi timestep (vs 3 stock ops), computing
  fv_t[i] = max_j fl(fl(fv_{t-1}[j] + T[i,j]) + obs_t[i])
which by monotonicity of fp rounding equals the reference's
fl(max_j fl(fv+T) + obs) bit-exactly.  Phases 2/3 (backpointer
extraction + backward traversal) are bulk-parallel stock ops.
"""
import sys
import numpy as np

if '/opt/trn_rl_repo' not in sys.path:
    sys.path.insert(0, '/opt/trn_rl_repo')

N_FULL, C, L = 64, 8, 32768
SEQ = 8          # sequences per core
NSTRIP = 16      # time strips per core for phases 2/3 (partition dim 128)
S = 16           # phase-3 chunk length
NCORES = 8
TBLK = 256       # phase-1 steps per block
PAGE = 9 * C     # 72 elems per step in the OT stream

_CACHE = {}

# ---------------------------------------------------------------- custom op
OP_NAME = "CRF_STEP_ANT"


def _register_crf_op():
    from concourse.dve_uop import (
        ENABLE, AluInp, AluOp, InpSel, OutPath, OutSel, Trigger, UopConfig,
        DveOpSpec,
    )
    from concourse.dve_spec import Spec, Src0, Src1
    from concourse import dve_ops

    if OP_NAME in dve_ops._SUB_OPCODE_FOR_NAME:
        return dve_ops._CRF_STEP_OP

    def _dp_tail(u):
        for k in range(2, 8):
            u.datapath_config[k].pass_through_alu()
        return u

    def _mk_steady():
        u = UopConfig()
        u.enable_input(InpSel.SRC_0, 0)
        u.enable_input(InpSel.SRC_1, 1)
        u.datapath_config[0].enable_alu(AluOp.ADD, AluInp.PREV_ALU_OUT,
                                        AluInp.PREV_DELAY_0)
        u.datapath_config[1].enable_alu(AluOp.MAX, AluInp.CURR_ALU_OUT,
                                        AluInp.PREV_ALU_OUT)
        u.require_inp0 = ENABLE
        u.require_inp1 = ENABLE
        u.repeat_count = C - 1
        u.trigger = (Trigger.SRC_TENSOR_DONE, Trigger.COUNT, Trigger.NONE)
        u.next_uop = (0, 2, 0)
        return _dp_tail(u)

    def _mk_obs():
        u = UopConfig()
        u.enable_input(InpSel.SRC_0, 0)
        u.enable_input(InpSel.SRC_1, 1)
        u.datapath_config[0].enable_alu(AluOp.BYPASS, AluInp.PREV_ALU_OUT)
        u.datapath_config[1].enable_alu(AluOp.ADD, AluInp.CURR_ALU_OUT,
                                        AluInp.PREV_ALU_OUT)
        u.enable_output(OutSel.ALU_OUT, OutPath.WR0_LO)
        u.require_inp0 = ENABLE
        u.require_inp1 = ENABLE
        u.repeat_count = 1
        u.trigger = (Trigger.SRC_TENSOR_DONE, Trigger.COUNT, Trigger.NONE)
        u.next_uop = (0, 3, 0)
        return _dp_tail(u)

    def _mk_reset():
        u = UopConfig()
        u.enable_input(InpSel.SRC_0, 0)
        u.enable_input(InpSel.SRC_1, 1)
        u.enable_input(InpSel.MAX_NEG, 2)
        u.datapath_config[0].enable_alu(AluOp.ADD, AluInp.PREV_ALU_OUT,
                                        AluInp.PREV_DELAY_0)
        u.datapath_config[0].pass_through_delay(1)
        u.datapath_config[1].enable_alu(AluOp.MAX, AluInp.PREV_DELAY_1,
                                        AluInp.PREV_ALU_OUT)
        u.require_inp0 = ENABLE
        u.require_inp1 = ENABLE
        u.repeat_count = 1
        u.trigger = (Trigger.SRC_TENSOR_DONE, Trigger.COUNT, Trigger.NONE)
        u.next_uop = (0, 1, 0)
        return _dp_tail(u)

    class CrfStepOp:
        name = OP_NAME
        subdim = False
        spec = Spec(body=Src0 + Src1, reference=lambda in0, in1: in0 + in1)

        def __init__(self):
            self._cache = {}

        def compile(self, ver):
            if ver not in self._cache:
                s = DveOpSpec(
                    name=self.name,
                    uops=[_mk_reset(), _mk_steady(), _mk_obs(), _mk_reset()],
                    opcode=dve_ops.get_dve_sub_opcode(self.name),
                    rd1_en=True,
                )
                s.validate(ver)
                self._cache[ver] = s
            return self._cache[ver]

    op = CrfStepOp()
    row = max(dve_ops._SUB_OPCODE_FOR_NAME.values()) + 1
    assert row < 0x20
    dve_ops._SUB_OPCODE_FOR_NAME[OP_NAME] = row
    dve_ops.OPS.append(op)
    dve_ops.CUSTOM_DVE_SPECS[OP_NAME] = op.spec
    dve_ops._CRF_STEP_OP = op
    return op


# ---------------------------------------------------------------- host side
def _shapes(L):
    STRIP = L // NSTRIP
    TB = min(128, STRIP)
    return dict(STRIP=STRIP, TB=TB, ROUNDS=STRIP // TB, KL=STRIP // S)


def _host_prep(observes_core, transitions, lengths_core, L):
    sh = _shapes(L)
    STRIP, KL = sh["STRIP"], sh["KL"]
    obs_t = np.ascontiguousarray(
        np.transpose(np.asarray(observes_core, np.float32), (0, 2, 1)))
    # scatter source: [s, t, i] contiguous + 2 blocks of zero pad
    obs_sc = np.zeros((SEQ, (L + 2 * TBLK) * C), np.float32)
    obs_sc[:, :L * C] = obs_t.reshape(SEQ, L * C)
    T = np.asarray(transitions, np.float32)
    # OT image for one block: page i of step t = [T[i,0..7], 0]
    ot1 = np.zeros((SEQ, TBLK * PAGE), np.float32)
    trow = np.zeros(PAGE, np.float32)
    for i in range(C):
        trow[i * 9:i * 9 + 8] = T[i, :]
    ot1[:, :] = np.tile(trow, TBLK)[None, :]
    lens = np.asarray(lengths_core).astype(np.float32)
    p = np.arange(128)
    return {
        "obs_sc": obs_sc,
        "ot_init": ot1,
        "trep": np.tile(T.reshape(1, C * C), (128, 1)).astype(np.float32),
        "wdesc": np.tile((C - np.arange(C, dtype=np.float32)).reshape(1, C),
                         (128, 1)),
        "tplane": ((p[:, None] // SEQ) * STRIP
                   + np.arange(STRIP)[None, :]).astype(np.float32),
        "len_col": lens[p % SEQ][:, None].astype(np.float32),
        "lenm1": (lens[p % SEQ][:, None] - 1.0).astype(np.float32),
        "einit1": np.tile((np.arange(C, dtype=np.float32)[:, None] + 1.0),
                          (1, KL)).reshape(1, C * KL).repeat(128, 0)
                    .astype(np.float32),
    }


def _host_post(path_dev, L):
    STRIP = L // NSTRIP
    return path_dev.reshape(NSTRIP, SEQ, STRIP).transpose(1, 0, 2).reshape(SEQ, L)


# ---------------------------------------------------------------- device
def _emit(tc, ins, outs, L):
    import concourse.bass as bass
    import concourse.mybir as mybir
    import bass_rust
    from concourse import dve_ops

    F32 = mybir.dt.float32
    I32 = mybir.dt.int32
    ALU = mybir.AluOpType
    AX = mybir.AxisListType

    crf_op = dve_ops._CRF_STEP_OP

    def v(ap, off, dims):
        return bass_rust.AP(tensor=ap.tensor, offset=ap.offset + off, ap=dims)

    nc = tc.nc
    sh = _shapes(L)
    STRIP, TB, ROUNDS, KL = sh["STRIP"], sh["TB"], sh["ROUNDS"], sh["KL"]
    G1 = min(8, KL)
    NG = KL // G1
    FLATN = (L + 1) * C
    NBLK = L // TBLK

    obs_d = ins["obs_sc"]
    ot_init_d = ins["ot_init"]
    trep_d = ins["trep"]
    wdesc_d = ins["wdesc"]
    tplane_d = ins["tplane"]
    len_d = ins["len_col"]
    lenm1_d = ins["lenm1"]
    einit1_d = ins["einit1"]
    path_d = outs["path"]

    fv_d = nc.dram_tensor("fv_scratch", [SEQ, FLATN], F32).ap()
    smap_d = nc.dram_tensor("smap_scratch", [128, C], F32).ap()
    estrip_d = nc.dram_tensor("estrip_scratch", [SEQ, NSTRIP], F32).ap()

    vec = nc.vector

    with tc.tile_pool(name="const", bufs=1) as cpool:
        trep = cpool.tile([128, C * C], F32)
        wdesc = cpool.tile([128, C], F32)
        tplane = cpool.tile([128, STRIP], F32)
        len_sb = cpool.tile([128, 1], F32)
        lenm1_sb = cpool.tile([128, 1], F32)
        nc.sync.dma_start(out=trep[:], in_=trep_d)
        nc.sync.dma_start(out=wdesc[:], in_=wdesc_d)
        nc.sync.dma_start(out=tplane[:], in_=tplane_d)
        nc.sync.dma_start(out=len_sb[:], in_=len_d)
        nc.sync.dma_start(out=lenm1_sb[:], in_=lenm1_d)

        # ============ phase 1: fused custom-op chain ============
        with tc.tile_pool(name="ph1", bufs=1) as pool:
            otA = pool.tile([SEQ, TBLK * PAGE], F32)
            otB = pool.tile([SEQ, TBLK * PAGE], F32)
            fvA = pool.tile([SEQ, TBLK * C + C], F32)
            fvB = pool.tile([SEQ, TBLK * C + C], F32)
            vec.memset(fvA[:], 0.0)
            vec.memset(fvB[:], 0.0)
            nc.sync.dma_start(out=otA[:], in_=ot_init_d)
            nc.sync.dma_start(out=otB[:], in_=ot_init_d)
            # fv_d col 0 = fv_init = 0
            nc.sync.dma_start(out=v(fv_d, 0, [[FLATN, SEQ], [1, C]]),
                              in_=fvA[:, 0:C])

            P = lambda t: t[:].ap[0]
            SC_SB = [[PAGE, TBLK], [9, C]]     # obs slots in an OT tile

            def steps(ot_t, fv_t, fv_prev_t):
                for t in range(TBLK):
                    src = (v(fv_prev_t[:], (TBLK - 1) * C,
                             [P(fv_prev_t), [0, C], [1, 9]]) if t == 0 else
                           v(fv_t[:], (t - 1) * C, [P(fv_t), [0, C], [1, 9]]))
                    nc.vector._custom_dve(
                        crf_op,
                        out=fv_t[:, t * C:(t + 1) * C],
                        in0=v(ot_t[:], t * PAGE, [P(ot_t), [9, C], [1, 9]]),
                        in1=src)

            # prologue: scatter obs for block 0 into otA
            nc.sync.dma_start(out=v(otA[:], C, [P(otA)] + SC_SB),
                              in_=obs_d[:, 0:TBLK * C])
            BB = TBLK * C  # elems per block in obs_sc AND in fv_d cols
            OBS_ROW = (L + 2 * TBLK) * C
            obs_p1 = v(obs_d, BB, [[OBS_ROW, SEQ], [1, OBS_ROW - BB]])
            obs_p2 = v(obs_d, 2 * BB, [[OBS_ROW, SEQ], [1, OBS_ROW - 2 * BB]])
            fv_lo = v(fv_d, C, [[FLATN, SEQ], [1, FLATN - C]])
            fv_hi = v(fv_d, C + BB, [[FLATN, SEQ], [1, FLATN - C - BB]])
            with tc.For_i(0, (NBLK - 2) * BB, 2 * BB,
                          hint_engines=(mybir.EngineType.DVE,),
                          staggered_reset=True) as iofs:
                nc.sync.dma_start(out=v(otB[:], C, [P(otB)] + SC_SB),
                                  in_=obs_p1[:, bass.ds(iofs, BB)])
                steps(otA, fvA, fvB)
                nc.sync.dma_start(out=fv_lo[:, bass.ds(iofs, BB)],
                                  in_=fvA[:, 0:BB])
                nc.sync.dma_start(out=v(otA[:], C, [P(otA)] + SC_SB),
                                  in_=obs_p2[:, bass.ds(iofs, BB)])
                steps(otB, fvB, fvA)
                nc.sync.dma_start(out=fv_hi[:, bass.ds(iofs, BB)],
                                  in_=fvB[:, 0:BB])
            # epilogue: last two blocks with static offsets
            lo = (NBLK - 2) * BB
            nc.sync.dma_start(out=v(otB[:], C, [P(otB)] + SC_SB),
                              in_=obs_d[:, lo + BB:lo + 2 * BB])
            steps(otA, fvA, fvB)
            nc.sync.dma_start(out=v(fv_d, C + lo, [[FLATN, SEQ], [1, BB]]),
                              in_=fvA[:, 0:BB])
            steps(otB, fvB, fvA)
            nc.sync.dma_start(out=v(fv_d, C + lo + BB,
                                    [[FLATN, SEQ], [1, BB]]),
                              in_=fvB[:, 0:BB])

        # ============ phase 2: backpointer extraction ============
        # bp stays in SBUF end-to-end: the 8 MB DRAM round-trip costs ~1 ms
        # of sync-queue descriptor generation alone.
        bp_pool = tc.alloc_tile_pool(name="bp", bufs=1)
        bp_sb = bp_pool.tile([128, STRIP * C], F32)
        fvpool = tc.alloc_tile_pool(name="ph2fv", bufs=3)
        with tc.tile_pool(name="ph2", bufs=1) as pool:
            for r in range(ROUNDS):
                off = r * TB * C
                fv_blk = fvpool.tile([128, (TB + 1) * C], F32, tag="fv",
                                     name=f"fvblk{r}")
                src_dims = [[STRIP * C, NSTRIP], [FLATN, SEQ],
                            [1, (TB + 1) * C]]
                # first loads jump the sync queue's end-of-phase-1
                # descriptor backlog via the idle scalar queue (static
                # offsets only -- register-offset DMAs hang there)
                dq = nc.scalar if r < 3 else nc.sync
                dq.dma_start(out=fv_blk[:], in_=v(fv_d, off, src_dims))

                P = lambda t: t[:].ap[0]
                sc2 = pool.tile([128, C * TB * C], F32, tag="sc")
                eq2 = pool.tile([128, C * TB * C], F32, tag="eq")
                vitr = pool.tile([128, C * TB], F32, tag="vitr")
                vec.tensor_tensor(
                    out=sc2[:],
                    in0=v(fv_blk[:], 0, [P(fv_blk), [0, C], [C, TB], [1, C]]),
                    in1=v(trep[:], 0, [P(trep), [C, C], [0, TB], [1, C]]),
                    op=ALU.add)
                vec.tensor_reduce(
                    out=vitr[:],
                    in_=v(sc2[:], 0, [P(sc2), [TB * C, C], [C, TB], [1, C]]),
                    axis=AX.X, op=ALU.max)
                vec.tensor_tensor(
                    out=eq2[:],
                    in0=v(sc2[:], 0, [P(sc2), [TB * C, C], [C, TB], [1, C]]),
                    in1=v(vitr[:], 0, [P(vitr), [TB, C], [1, TB], [0, C]]),
                    op=ALU.is_equal)
                vec.tensor_tensor(
                    out=eq2[:],
                    in0=v(eq2[:], 0, [P(eq2), [TB * C, C], [C, TB], [1, C]]),
                    in1=v(wdesc[:], 0, [P(wdesc), [0, C], [0, TB], [1, C]]),
                    op=ALU.mult)
                bpw = pool.tile([128, C * TB], F32, tag="bpw")
                vec.tensor_reduce(
                    out=bpw[:],
                    in_=v(eq2[:], 0, [P(eq2), [TB * C, C], [C, TB], [1, C]]),
                    axis=AX.X, op=ALU.max)
                bp1 = pool.tile([128, C * TB], F32, tag="bp1")
                vec.tensor_scalar(out=bp1[:], in0=bpw[:], scalar1=-1.0,
                                  scalar2=9.0, op0=ALU.mult, op1=ALU.add)

                fm = pool.tile([128, TB], F32, tag="fm")
                vec.tensor_reduce(
                    out=fm[:],
                    in_=v(fv_blk[:], C, [P(fv_blk), [C, TB], [1, C]]),
                    axis=AX.X, op=ALU.max)
                eqn = pool.tile([128, TB * C], F32, tag="eqn")
                vec.tensor_tensor(
                    out=eqn[:],
                    in0=v(fv_blk[:], C, [P(fv_blk), [C, TB], [1, C]]),
                    in1=v(fm[:], 0, [P(fm), [1, TB], [0, C]]),
                    op=ALU.is_equal)
                vec.tensor_tensor(
                    out=eqn[:],
                    in0=v(eqn[:], 0, [P(eqn), [C, TB], [1, C]]),
                    in1=v(wdesc[:], 0, [P(wdesc), [0, TB], [1, C]]),
                    op=ALU.mult)
                mn = pool.tile([128, TB], F32, tag="mn")
                vec.tensor_reduce(
                    out=mn[:],
                    in_=v(eqn[:], 0, [P(eqn), [C, TB], [1, C]]),
                    axis=AX.X, op=ALU.max)
                en1 = pool.tile([128, TB], F32, tag="en1")
                vec.tensor_scalar(out=en1[:], in0=mn[:], scalar1=-1.0,
                                  scalar2=9.0, op0=ALU.mult, op1=ALU.add)
                endsel = pool.tile([128, TB], F32, tag="endsel")
                tmp = pool.tile([128, TB], F32, tag="tmpsel")
                for j in range(C):
                    dst = endsel if j == 0 else tmp
                    vec.scalar_tensor_tensor(
                        out=dst[:], in0=en1[:], scalar=float(j + 1),
                        in1=bp1[:, j * TB:(j + 1) * TB],
                        op0=ALU.is_equal, op1=ALU.mult)
                    if j > 0:
                        vec.tensor_tensor(out=endsel[:], in0=endsel[:],
                                          in1=tmp[:], op=ALU.max)
                atm = pool.tile([128, TB], F32, tag="atm")
                vec.tensor_scalar(out=atm[:], in0=tplane[:, r * TB:(r + 1) * TB],
                                  scalar1=lenm1_sb[:], scalar2=None,
                                  op0=ALU.is_equal)
                dsel = pool.tile([128, TB * C], F32, tag="dsel")
                bp1_tn = v(bp1[:], 0, [P(bp1), [1, TB], [TB, C]])
                vec.tensor_tensor(
                    out=dsel[:],
                    in0=v(endsel[:], 0, [P(endsel), [1, TB], [0, C]]),
                    in1=bp1_tn, op=ALU.subtract)
                vec.tensor_tensor(
                    out=dsel[:],
                    in0=v(dsel[:], 0, [P(dsel), [C, TB], [1, C]]),
                    in1=v(atm[:], 0, [P(atm), [1, TB], [0, C]]),
                    op=ALU.mult)
                vec.tensor_tensor(out=bp_sb[:, off:off + TB * C],
                                  in0=bp1_tn, in1=dsel[:], op=ALU.add)

        fvpool.release()
        tc.strict_bb_all_engine_barrier()

        # ============ phase 3: chunked backward ============
        with tc.tile_pool(name="ph3", bufs=1) as pool:
            P = lambda t: t[:].ap[0]
            bp_strip = bp_sb
            einit1 = pool.tile([128, C * KL], F32)
            nc.sync.dma_start(out=einit1[:], in_=einit1_d)
            cand1 = pool.tile([128, C * KL * S], F32)
            acc = pool.tile([128, C * KL], F32)
            tmp = pool.tile([128, C * KL], F32)

            def cand_col(tl):
                return v(cand1[:], tl, [P(cand1), [KL * S, C], [S, KL]])

            for tl in range(S - 1, -1, -1):
                if tl == S - 1:
                    prev = v(einit1[:], 0, [P(einit1), [KL, C], [1, KL]])
                else:
                    prev = cand_col(tl + 1)
                for j in range(C):
                    dst = acc[:] if j == 0 else tmp[:]
                    vec.scalar_tensor_tensor(
                        out=dst, in0=prev, scalar=float(j + 1),
                        in1=v(bp_strip[:], tl * C + j,
                              [P(bp_strip), [0, C], [S * C, KL]]),
                        op0=ALU.is_equal, op1=ALU.mult)
                    if j > 0:
                        out_ap = cand_col(tl) if j == C - 1 else acc[:]
                        vec.tensor_tensor(out=out_ap, in0=acc[:], in1=tmp[:],
                                          op=ALU.max)

            m1a = pool.tile([128, C * NG], F32)
            m1b = pool.tile([128, C * NG], F32)
            t1 = pool.tile([128, C * NG], F32)
            a1 = pool.tile([128, C * NG], F32)
            vec.tensor_copy(out=m1a[:],
                            in_=v(einit1[:], 0, [P(einit1), [KL, C], [G1, NG]]))
            cur, nxt = m1a, m1b
            for kk in range(G1 - 1, -1, -1):
                for j in range(C):
                    dst = a1[:] if j == 0 else t1[:]
                    vec.scalar_tensor_tensor(
                        out=dst, in0=cur[:], scalar=float(j + 1),
                        in1=v(cand1[:], j * KL * S + kk * S,
                              [P(cand1), [0, C], [G1 * S, NG]]),
                        op0=ALU.is_equal, op1=ALU.mult)
                    if j > 0:
                        out_ap = nxt[:] if j == C - 1 else a1[:]
                        vec.tensor_tensor(out=out_ap, in0=a1[:], in1=t1[:],
                                          op=ALU.max)
                cur, nxt = nxt, cur
            m1 = cur

            msa = pool.tile([128, C], F32)
            msb = pool.tile([128, C], F32)
            t2 = pool.tile([128, C], F32)
            a2 = pool.tile([128, C], F32)
            vec.tensor_copy(out=msa[:], in_=v(einit1[:], 0,
                                              [P(einit1), [KL, C], [1, 1]]))
            cur2, nxt2 = msa, msb
            for g in range(NG - 1, -1, -1):
                for j in range(C):
                    dst = a2[:] if j == 0 else t2[:]
                    vec.scalar_tensor_tensor(
                        out=dst, in0=cur2[:], scalar=float(j + 1),
                        in1=v(m1[:], j * NG + g, [P(m1), [0, C], [0, 1]]),
                        op0=ALU.is_equal, op1=ALU.mult)
                    if j > 0:
                        out_ap = nxt2[:] if j == C - 1 else a2[:]
                        vec.tensor_tensor(out=out_ap, in0=a2[:], in1=t2[:],
                                          op=ALU.max)
                cur2, nxt2 = nxt2, cur2
            nc.sync.dma_start(out=smap_d[:], in_=cur2[:])
            tc.strict_bb_all_engine_barrier()

            smap_t = pool.tile([SEQ, NSTRIP * C], F32)
            nc.sync.dma_start(out=smap_t[:],
                              in_=v(smap_d, 0, [[C, SEQ], [C * SEQ, NSTRIP],
                                                [1, C]]))
            state = pool.tile([SEQ, 1], F32)
            sacc = pool.tile([SEQ, 1], F32)
            stmp = pool.tile([SEQ, 1], F32)
            estrip = pool.tile([SEQ, NSTRIP], F32)
            vec.memset(state[:], 1.0)
            for sg in range(NSTRIP - 1, -1, -1):
                vec.tensor_copy(out=estrip[:, sg:sg + 1], in_=state[:])
                for j in range(C):
                    dst = sacc if j == 0 else stmp
                    vec.scalar_tensor_tensor(
                        out=dst[:], in0=state[:], scalar=float(j + 1),
                        in1=smap_t[:, sg * C + j:sg * C + j + 1],
                        op0=ALU.is_equal, op1=ALU.mult)
                    if j > 0:
                        out_ap = state[:] if j == C - 1 else sacc[:]
                        vec.tensor_tensor(out=out_ap, in0=sacc[:], in1=stmp[:],
                                          op=ALU.max)
            nc.sync.dma_start(out=estrip_d, in_=estrip[:])
            tc.strict_bb_all_engine_barrier()
            eseed = pool.tile([128, 1], F32)
            nc.sync.dma_start(out=eseed[:],
                              in_=v(estrip_d, 0, [[1, NSTRIP], [NSTRIP, SEQ],
                                                  [1, 1]]))

            eg = pool.tile([128, NG], F32)
            st2 = pool.tile([128, 1], F32)
            d2a = pool.tile([128, 1], F32)
            d2t = pool.tile([128, 1], F32)
            vec.tensor_copy(out=st2[:], in_=eseed[:])
            for g in range(NG - 1, -1, -1):
                vec.tensor_copy(out=eg[:, g:g + 1], in_=st2[:])
                for j in range(C):
                    dst = d2a if j == 0 else d2t
                    vec.scalar_tensor_tensor(
                        out=dst[:], in0=st2[:], scalar=float(j + 1),
                        in1=v(m1[:], j * NG + g, [P(m1), [0, 1]]),
                        op0=ALU.is_equal, op1=ALU.mult)
                    if j > 0:
                        out_ap = st2[:] if j == C - 1 else d2a[:]
                        vec.tensor_tensor(out=out_ap, in0=d2a[:], in1=d2t[:],
                                          op=ALU.max)

            ek = pool.tile([128, KL], F32)
            st3 = pool.tile([128, NG], F32)
            d1a = pool.tile([128, NG], F32)
            d1t = pool.tile([128, NG], F32)
            vec.tensor_copy(out=st3[:], in_=eg[:])
            for kk in range(G1 - 1, -1, -1):
                vec.tensor_copy(out=v(ek[:], kk, [P(ek), [G1, NG]]), in_=st3[:])
                for j in range(C):
                    dst = d1a if j == 0 else d1t
                    vec.scalar_tensor_tensor(
                        out=dst[:], in0=st3[:], scalar=float(j + 1),
                        in1=v(cand1[:], j * KL * S + kk * S,
                              [P(cand1), [G1 * S, NG]]),
                        op0=ALU.is_equal, op1=ALU.mult)
                    if j > 0:
                        out_ap = st3[:] if j == C - 1 else d1a[:]
                        vec.tensor_tensor(out=out_ap, in0=d1a[:], in1=d1t[:],
                                          op=ALU.max)

            acc2 = pool.tile([128, STRIP], F32)
            tsel = pool.tile([128, STRIP], F32)
            for e in range(C):
                dst = acc2 if e == 0 else tsel
                vec.scalar_tensor_tensor(
                    out=dst[:],
                    in0=v(ek[:], 0, [P(ek), [1, KL], [0, S]]),
                    scalar=float(e + 1),
                    in1=v(cand1[:], e * KL * S, [P(cand1), [S, KL], [1, S]]),
                    op0=ALU.is_equal, op1=ALU.mult)
                if e > 0:
                    vec.tensor_tensor(out=acc2[:], in0=acc2[:], in1=tsel[:],
                                      op=ALU.max)
            mask = pool.tile([128, STRIP], F32)
            vec.tensor_scalar(out=mask[:], in0=tplane[:], scalar1=len_sb[:],
                              scalar2=None, op0=ALU.is_lt)
            vec.tensor_tensor(out=acc2[:], in0=acc2[:], in1=mask[:],
                              op=ALU.mult)
            vec.tensor_scalar(out=acc2[:], in0=acc2[:], scalar1=-1.0,
                              scalar2=None, op0=ALU.add)
            path_i = pool.tile([128, STRIP], I32)
            vec.tensor_copy(out=path_i[:], in_=acc2[:])
            nc.sync.dma_start(out=path_d, in_=path_i[:])
        bp_pool.release()


def _build(L):
    import concourse.bacc as bacc
    import concourse.mybir as mybir
    from concourse import tile

    _register_crf_op()
    sh = _shapes(L)
    nc = bacc.Bacc("TRN2", target_bir_lowering=False, debug=False,
                   num_devices=NCORES)
    F32 = mybir.dt.float32
    ins_aps = {
        "obs_sc": nc.dram_tensor("obs_sc", [SEQ, (L + 2 * TBLK) * C], F32,
                                 kind="ExternalInput").ap(),
        "ot_init": nc.dram_tensor("ot_init", [SEQ, TBLK * PAGE], F32,
                                  kind="ExternalInput").ap(),
        "trep": nc.dram_tensor("trep", [128, C * C], F32,
                               kind="ExternalInput").ap(),
        "wdesc": nc.dram_tensor("wdesc", [128, C], F32,
                                kind="ExternalInput").ap(),
        "tplane": nc.dram_tensor("tplane", [128, sh["STRIP"]], F32,
                                 kind="ExternalInput").ap(),
        "len_col": nc.dram_tensor("len_col", [128, 1], F32,
                                  kind="ExternalInput").ap(),
        "lenm1": nc.dram_tensor("lenm1", [128, 1], F32,
                                kind="ExternalInput").ap(),
        "einit1": nc.dram_tensor("einit1", [128, C * sh["KL"]], F32,
                                 kind="ExternalInput").ap(),
    }
    outs_aps = {"path": nc.dram_tensor("path", [128, sh["STRIP"]],
                                       mybir.dt.int32,
                                       kind="ExternalOutput").ap()}
    with tile.TileContext(nc) as tc:
        _emit(tc, ins_aps, outs_aps, L)
    nc.compile()
    return nc


def kernel(observes, transitions, lengths):
    from concourse.bass_utils import run_bass_kernel_spmd

    observes = np.asarray(observes, np.float32)
    transitions = np.asarray(transitions, np.float32)
    lengths_np = np.asarray(lengths)
    L = observes.shape[2]

    if L not in _CACHE:
        _CACHE[L] = _build(L)
    nc = _CACHE[L]

    in_maps = [
        _host_prep(observes[SEQ * c:SEQ * (c + 1)], transitions,
                   lengths_np[SEQ * c:SEQ * (c + 1)], L)
        for c in range(NCORES)
    ]
    res = run_bass_kernel_spmd(nc, in_maps, core_ids=list(range(NCORES)))
    out = np.concatenate(
        [_host_post(res.results[c]["path"], L) for c in range(NCORES)], 0)
    return out.astype(np.int32)
